# revision 1
# baseline (speedup 1.0000x reference)
"""Distributed attention kernel for TRN2 (8 NeuronCores).

Computes: softmax(sqrt(Dqk) * (x@Wq.T) @ (x@Wk.T).T) @ (x@Wv.T)
for x [8192, 1024], Wq/Wk/Wv [256, 1024], out [8192, 256].

Sharding: rows of x across 8 cores (sequence parallel), weights
replicated; K^T/V AllGathered; flash-style attention over each core's
1024 Q rows. ~258-270us (v1 baseline: ~298us, single-AG fully serial).

Key structure (hard-won; see git/transcript for the experiments):
  - q^T/k^T are f16 (adds ~1e-3 rel err; gate is 2e-2): halves the AG
    payload and kt_full SBUF.
  - TWO AllGathers: AG1 = K^T (1 MB in -> 4 MB out), triggered right
    after the k-projection (x^T for each 512-row half is interleaved
    with k-proj so staging starts ASAP); AG2 = V, triggered after
    v-proj. They serialize on the CC engine but AG1 unblocks ch>=1
    SCORES ~35us before AG2 finishes; only PV needs V.
  - Tile rotates hardware DMAs over 8 completion semaphores in EMISSION
    order, and the scheduler may hoist AG-gated DMAs ahead of emission
    order within an engine stream. Assembly (anchor + per-slot DMAs) is
    therefore emitted inside tc.tile_wait_until(1.0) AND placed on the
    SYNC engine, which is idle when AG1 completes (scalar still owes
    ~30us of exps/evictions at that point). anchor = static-offset
    pack_*_ag read that carries the collective wait for the
    dynamic-offset (cc_rank-addressed) slot DMAs Tile cannot track.
  - DMA xbar (dma_start_transpose) is hardware-serialized against
    in-flight collectives, and each DMA_TRANSPOSE costs ~1.3us of
    issuing-engine time. So ch0/ch1 (which overlap AG1/AG2) transpose
    P on the PE instead (bf16 identity transpose into bitcast slices of
    the scores-PSUM ring, lagged one chunk so the in-order PE never
    stalls on the exp chain); ch2+ use the xbar on the sync engine.
    (ch2 on the PE was tried: it steals scores-ring slots and kills the
    exp run-ahead that fills the AG2 window -> worse.)
  - exp bias = RUNNING max (running-min chain over negated chunk
    maxima), so no per-chunk beta on P and the accumulator merge is one
    scalar_tensor_tensor (acc = acc*gamma + po) on DVE.
  - V carries a constant ones-column (memset locally, not AllGathered):
    PV accumulates the softmax denominator with all gamma corrections
    for free; final normalize = reciprocal of that column. No
    accum_out, no alpha/sigma bookkeeping.
  - PV lags scores by LAG=28 chunks (ch1 PVs must not reach the
    in-order PE before AG2 lands their V slots); from ch5 on, two PVs
    retire per chunk so the final drain is ~10 PVs, not 28.
  - Steady state is ~1.9-2.2us/chunk, DVE-bound (reduce_max 1.25us +
    merge 0.5us); ACT ~75% (exp 1.1us + gamma); PE runs mostly at mid
    pstate. Run-to-run +-15us from collective/barrier skew.
  - CLOSED LEADS: (a) f16 projections with xbar x^T/W^T: numerically
    fine (6e-3) once transpose OUTPUT slices are contiguous (strided
    3-D dma_start_transpose outputs corrupt silently on HW!), but a
    [128,1024] xbar transpose is 1024 x 256B packets and 14 of them
    are ~27us of packet-bound DMA wall-time serialized before k-proj
    and AG1 -> net LOSS vs PE transposes (311us measured; see
    kernel_exp.py in the session transcript). (b) paired-chunk PV
    merge: exact math, but holding both scores tiles until the
    shared-bias exps serializes the 3-deep PSUM ring (322us). (c)
    bf16 projections: 3.5% err, over the gate.

Per 128-row i-tile: scores chunks in PSUM (f16 matmul, 2xLDW/chunk),
chunk row-max on DVE, exp on ACT (PSUM -> SBUF bf16, running-max bias),
P^T (PE or xbar), PV matmul bf16 over [V | 1], one-op merge, normalize,
DMA out.
"""

import numpy as np

import concourse.bacc as bacc
import concourse.bass as bass
import concourse.mybir as mybir
import concourse.tile as tile
from concourse.bass_utils import run_bass_kernel_spmd
from concourse.masks import make_identity

F32 = mybir.dt.float32
F32R = mybir.dt.float32r
BF16 = mybir.dt.bfloat16
F16 = mybir.dt.float16

N_CORES = 8
N, D, DQK, DV = 8192, 1024, 256, 256
P = 128
CHUNK = 1024  # scores chunk width (2 PSUM banks)


def build(n=N, d=D, dqk=DQK, dv=DV, ncores=N_CORES):
    nsh = n // ncores
    IT = nsh // P       # i-tiles per core
    KT = d // P         # contract tiles for projections
    CT = dqk // P       # dqk tiles
    JT = n // P         # j tiles for PV
    chunk = min(CHUNK, nsh)
    assert chunk == nsh, "rotation requires one chunk per rank"
    NCH = n // chunk    # score chunks per row == ncores
    W512 = min(512, chunk)
    NN = chunk // W512  # W512-wide matmuls per chunk
    scale = float(np.sqrt(dqk))

    # Two packed AG buffers, rows of nsh u16 (2KB):
    #   pack_k: k^T as [dqk, nsh] f16 — AllGathered FIRST (gates scores)
    #   pack_v: v as [nsh, dv] bf16; partition p's [IT, dv] slab =
    #           rows 2p, 2p+1 — AllGathered second (gates PV only)
    U16 = mybir.dt.uint16

    nc = bacc.Bacc(None, target_bir_lowering=False, num_devices=ncores)

    x_ext = nc.declare_dram_parameter("x", [nsh, d], F32, isOutput=False)
    wq_ext = nc.declare_dram_parameter("Wq", [dqk, d], F32, isOutput=False)
    wk_ext = nc.declare_dram_parameter("Wk", [dqk, d], F32, isOutput=False)
    wv_ext = nc.declare_dram_parameter("Wv", [dv, d], F32, isOutput=False)
    out_ext = nc.declare_dram_parameter("out", [nsh, dv], F32, isOutput=True)

    groups = [list(range(ncores))]

    with tile.TileContext(nc) as tc:
        with (
            tc.tile_pool(name="persist", bufs=1) as pp,
            tc.tile_pool(name="dramp", bufs=1, space="DRAM") as dp,
        ):
            pack_k = dp.tile([dqk, nsh], U16, name="pack_k")
            pack_k_ag = dp.tile(
                [ncores * dqk, nsh], U16, addr_space="Shared",
                name="pack_k_ag",
            )
            pack_v = dp.tile([2 * P, nsh], U16, name="pack_v")
            pack_v_ag = dp.tile(
                [ncores * 2 * P, nsh], U16, addr_space="Shared",
                name="pack_v_ag",
            )
            dvp = dv + 1  # extra ones-column: PV accumulates the
            # softmax denominator alongside the numerator (gamma
            # corrections apply to both for free)
            qt_s = pp.tile([P, CT, nsh], F16, tag="qt")
            kt_loc = pp.tile([P, CT, nsh], F16, tag="ktloc")
            v_loc = pp.tile([P, IT, dvp], BF16, tag="vloc")
            nc.vector.memset(v_loc[:, :, dv:dvp], 1.0)
            ident = pp.tile([P, P], F32, tag="ident")
            make_identity(nc, ident[:])
            ident_b = pp.tile([P, P], BF16, tag="identb")
            make_identity(nc, ident_b[:])

            # ================= Phase A =================
            with (
                tc.tile_pool(name="phA", bufs=1) as pa,
                tc.tile_pool(name="phA_psum", bufs=1, space="PSUM") as paps,
            ):
                x_nat = pa.tile([P, IT, d], F32, tag="xnat")
                w_nat = pa.tile([P, 3 * CT, d], F32, tag="wnat")
                w_exts = {0: wq_ext, 1: wk_ext, 2: wv_ext}

                def w_dma(wi):
                    nc.scalar.dma_start(
                        w_nat[:, wi * CT:(wi + 1) * CT, :],
                        w_exts[wi].ap().rearrange("(ct p) d -> p ct d", p=P),
                    )

                def x_dma(it, eng):
                    eng.dma_start(
                        x_nat[:, it, :],
                        x_ext.ap().rearrange("(it p) d -> p it d", p=P)[:, it, :],
                    )

                # scalar queue: Wk FIRST (it gates the Wk transposes ->
                # k-proj -> AG1 path); x evens go on sync in parallel
                w_dma(1)
                for it in range(0, IT, 2):
                    x_dma(it, nc.sync)
                for it in range(1, IT, 2):
                    x_dma(it, nc.scalar)
                w_dma(2)
                w_dma(0)

                xt_s = pa.tile([P, KT, nsh], F32R, tag="xt")
                wt_s = pa.tile([P, 3 * KT, dqk], F32R, tag="wt")

                ei = 0  # eviction engine alternator

                def evict(dst, src):
                    nonlocal ei
                    ei += 1
                    if ei % 2:
                        nc.vector.tensor_copy(dst, src)
                    else:
                        nc.scalar.copy(dst, src)

                # interleave x^T with the k-projection path so k staging
                # (which gates the AllGather) starts as early as possible
                ich_w = min(512, nsh)

                def x_transposes(it_range):
                    for it in it_range:
                        for kt in range(KT):
                            tp = paps.tile([P, P], F32, tag="tp", bufs=4)
                            nc.tensor.transpose(
                                tp[:], x_nat[:, it, kt * P:(kt + 1) * P], ident[:]
                            )
                            evict(xt_s[:, kt, it * P:(it + 1) * P], tp[:])

                def w_transposes(wi):
                    for kt in range(KT):
                        for ct in range(CT):
                            tp = paps.tile([P, P], F32, tag="tp", bufs=4)
                            nc.tensor.transpose(
                                tp[:],
                                w_nat[:, wi * CT + ct, kt * P:(kt + 1) * P],
                                ident[:],
                            )
                            evict(wt_s[:, wi * KT + kt, ct * P:(ct + 1) * P], tp[:])

                def k_proj(ich):
                    for ct in range(CT):
                        psk = paps.tile([P, ich_w], F32, tag="pqk", bufs=2)
                        for kt in range(KT):
                            nc.tensor.matmul(
                                psk[:],
                                wt_s[:, 1 * KT + kt, ct * P:(ct + 1) * P],
                                xt_s[:, kt, ich * ich_w:(ich + 1) * ich_w],
                                start=(kt == 0),
                                stop=(kt == KT - 1),
                            )
                        nc.vector.tensor_copy(
                            kt_loc[:, ct, ich * ich_w:(ich + 1) * ich_w],
                            psk[:],
                        )
                    # stage this ich half immediately (AG waits on staging)
                    nc.sync.dma_start(
                        pack_k[:].bitcast(F16).rearrange(
                            "(ct p) i -> p ct i", p=P
                        )[:, :, ich * ich_w:(ich + 1) * ich_w],
                        kt_loc[:, :, ich * ich_w:(ich + 1) * ich_w],
                    )

                # k_proj(ich) only needs x^T of i-rows in that half:
                # interleave so AG1 triggers as early as possible
                ipw = ich_w // P
                x_transposes(range(ipw))
                w_transposes(1)           # Wk
                k_proj(0)
                x_transposes(range(ipw, IT))
                k_proj(1)
                # K AllGather fires as soon as k^T is staged; scores for
                # ch>=1 depend only on this one
                nc.gpsimd.collective_compute(
                    "AllGather",
                    mybir.AluOpType.bypass,
                    replica_groups=groups,
                    ins=[pack_k[:].opt()],
                    outs=[pack_k_ag[:].opt()],
                )
                w_transposes(2)           # Wv
                # v projections
                for it in range(IT):
                    psv = paps.tile([P, dv], F32, tag="psv", bufs=2)
                    for kt in range(KT):
                        nc.tensor.matmul(
                            psv[:],
                            xt_s[:, kt, it * P:(it + 1) * P],
                            wt_s[:, 2 * KT + kt, :dqk],
                            start=(kt == 0),
                            stop=(kt == KT - 1),
                        )
                    nc.vector.tensor_copy(v_loc[:, it, :dv], psv[:])
                # v packed: partition p's [IT, dv] slab -> rows 2p, 2p+1
                # (the ones column stays local; receivers memset their own)
                nc.sync.dma_start(
                    pack_v[:].bitcast(BF16).rearrange(
                        "(p r) i -> p (r i)", p=P
                    ),
                    v_loc[:, :, :dv],
                )
                # V AllGather (runs after the K one on the CC stream)
                nc.gpsimd.collective_compute(
                    "AllGather",
                    mybir.AluOpType.bypass,
                    replica_groups=groups,
                    ins=[pack_v[:].opt()],
                    outs=[pack_v_ag[:].opt()],
                )

                # Wq transposes + q^T projection (overlap the AllGather).
                # ich outer so q rows 0-511 finish first and ch0 scores
                # can start while the second half projects.
                w_transposes(0)
                for ich in range(nsh // ich_w):
                    for ct in range(CT):
                        psq = paps.tile([P, ich_w], F32, tag="pqk", bufs=2)
                        for kt in range(KT):
                            nc.tensor.matmul(
                                psq[:],
                                wt_s[:, 0 * KT + kt, ct * P:(ct + 1) * P],
                                xt_s[:, kt, ich * ich_w:(ich + 1) * ich_w],
                                start=(kt == 0),
                                stop=(kt == KT - 1),
                            )
                        nc.vector.tensor_scalar_mul(
                            qt_s[:, ct, ich * ich_w:(ich + 1) * ich_w],
                            psq[:],
                            scale,
                        )

            phb_cm = tc.tile_pool(name="phB", bufs=1)
            phb = phb_cm.__enter__()
            kt_full = phb.tile([P, CT, n], F16, tag="ktf", name="kt_full")
            v_s = phb.tile([P, JT, dvp], BF16, tag="vs", name="v_s")
            nc.vector.memset(v_s[:, :, dv:dvp], 1.0)

            pag_h = pack_k_ag[:].bitcast(F16)
            pag_b = pack_v_ag[:].bitcast(BF16)

            def assemble_v(s, rk_sync):
                # slot s holds rank (my_rank + s) % ncores; V must be on
                # the sync queue (corrupts on any other - empirical)
                vrow = ((rk_sync + s) % ncores) * 2 * P
                nc.sync.dma_start(
                    v_s[:, s * IT:(s + 1) * IT, :dv],
                    pag_b[bass.ds(vrow, 2 * P), :].rearrange(
                        "(p r) i -> p (r i)", p=P
                    ),
                )

            def assemble_kt(s, rk_kt):
                row = ((rk_kt + s) % ncores) * dqk
                nc.sync.dma_start(
                    kt_full[:, :, s * nsh:(s + 1) * nsh],
                    pag_h[bass.ds(row, dqk), :].rearrange(
                        "(ct p) i -> p ct i", p=P
                    ),
                )

            # ================= Main attention loop =================
            # Streaming flash-attention: each 1024-wide chunk flows
            # MM -> row-max(Pool) -> exp(ACT, running-max bias) -> P^T
            # transpose -> PV -> one-op accumulator merge.
            with (
                tc.tile_pool(name="mainA", bufs=8) as ma,
                tc.tile_pool(name="chunkp", bufs=2) as cp,
                tc.tile_pool(name="scores_psum", bufs=3, space="PSUM") as sps,
                tc.tile_pool(name="out_psum", bufs=2, space="PSUM") as ops,
            ):
                JPC = chunk // P  # j-tiles per chunk (== IT)

                NTOT = IT * NCH
                stats = []
                for it in range(IT):
                    st = {
                        "mneg": ma.tile([P, NCH], F32, tag="mneg", name="mneg"),
                        "nmr": ma.tile([P, NCH], F32, tag="nmr", name="nmr"),
                        "rinv": ma.tile([P, 1], F32, tag="rinv", name="rinv"),
                        "gam": ma.tile([P, NCH], F32, tag="gam", name="gam"),
                        "acc": ma.tile([P, dvp], F32, tag="acc", name="acc"),
                    }
                    stats.append(st)

                import collections
                pend = collections.deque()  # (k, pt_c) with deep PV lag

                def do_scores(k):
                    ch, it = divmod(k, IT)
                    st = stats[it]
                    ps = sps.tile([P, chunk], F32, tag="s", name="ps")
                    for ct in range(CT):
                        for nn in range(NN):
                            nc.tensor.matmul(
                                ps[:, nn * W512:(nn + 1) * W512],
                                qt_s[:, ct, it * P:(it + 1) * P],
                                (kt_loc[:, ct, nn * W512:(nn + 1) * W512]
                                 if ch == 0 else
                                 kt_full[
                                     :, ct,
                                     ch * chunk + nn * W512:
                                     ch * chunk + (nn + 1) * W512,
                                 ]),
                                start=(ct == 0),
                                stop=(ct == CT - 1),
                                skip_group_check=True,
                            )
                    nc.vector.reduce_max(
                        st["mneg"][:, ch:ch + 1], ps[:],
                        axis=mybir.AxisListType.X, negate=True,
                    )
                    if ch > 0:
                        nc.vector.tensor_tensor(
                            st["nmr"][:, ch:ch + 1], st["nmr"][:, ch - 1:ch],
                            st["mneg"][:, ch:ch + 1], op=mybir.AluOpType.min,
                        )
                    else:
                        nc.vector.tensor_copy(st["nmr"][:, :1], st["mneg"][:, :1])
                    p_c = cp.tile([P, chunk], BF16, tag="p", name="p_c", bufs=16)
                    # bias = running max -> P is exp(s - m_run(ch)); no
                    # beta, and no accum_out: the denominator comes from
                    # the PV ones-column
                    nc.scalar.activation(
                        p_c[:], ps[:],
                        mybir.ActivationFunctionType.Exp,
                        bias=st["nmr"][:, ch:ch + 1],
                        scale=1.0,
                    )
                    if ch < 2:
                        # xbar transposes are serialized against in-flight
                        # collectives; ch0/ch1 (which overlap the K and V
                        # AllGathers) must transpose on the PE instead
                        return p_c
                    pt_c = cp.tile([P, JPC, P], BF16, tag="pt", name="pt_c", bufs=30)
                    # DMA_TRANSPOSE burns ~1.3us of ISSUING-ENGINE time;
                    # keep them all on the otherwise-idle sync engine
                    nc.sync.dma_start_transpose(pt_c[:], p_c[:])
                    return pt_c

                def do_tr_pe(p_c_t):
                    # P^T via PE for ch0 (PSUM is full: borrow a scores-
                    # ring tile and pack the 8 bf16 transpose outputs into
                    # its first half via bitcast slices)
                    pt_c = cp.tile([P, JPC, P], BF16, tag="pt", name="pt_c", bufs=30)
                    ps_tr = sps.tile([P, chunk], F32, tag="s", name="ps_tr")
                    for j2 in range(JPC):
                        tpp = ps_tr[:, j2 * 64:(j2 + 1) * 64].bitcast(BF16)
                        nc.tensor.transpose(
                            tpp, p_c_t[:, j2 * P:(j2 + 1) * P], ident_b[:]
                        )
                        if j2 % 2:
                            nc.vector.tensor_copy(pt_c[:, j2, :], tpp)
                        else:
                            nc.scalar.copy(pt_c[:, j2, :], tpp)
                    return pt_c

                def do_pv(k, pt_c):
                    ch, it = divmod(k, IT)
                    st = stats[it]
                    po = ops.tile([P, dvp], F32, tag="po", name="po")
                    for j2 in range(JPC):
                        nc.tensor.matmul(
                            po[:], pt_c[:, j2, :],
                            (v_loc[:, j2, :] if ch == 0 else
                             v_s[:, ch * JPC + j2, :]),
                            start=(j2 == 0), stop=(j2 == JPC - 1),
                        )
                    if ch == 0:
                        nc.vector.tensor_copy(st["acc"][:], po[:])
                    else:
                        # gamma = exp(m_run(ch-1) - m_run(ch))
                        nc.scalar.activation(
                            st["gam"][:, ch:ch + 1], st["nmr"][:, ch - 1:ch],
                            mybir.ActivationFunctionType.Exp,
                            bias=st["nmr"][:, ch:ch + 1], scale=-1.0,
                        )
                        # acc = acc*gamma + po (P already carries beta via
                        # the running-max exp bias)
                        nc.vector.scalar_tensor_tensor(
                            st["acc"][:], st["acc"][:], st["gam"][:, ch:ch + 1],
                            po[:],
                            op0=mybir.AluOpType.mult,
                            op1=mybir.AluOpType.add,
                        )
                    if ch == NCH - 1:
                        # denominator rode along in the ones-column
                        nc.vector.reciprocal(
                            st["rinv"][:], st["acc"][:, dv:dvp]
                        )
                        nc.vector.tensor_scalar_mul(
                            st["acc"][:, :dv], st["acc"][:, :dv], st["rinv"][:]
                        )
                        nc.sync.dma_start(
                            out_ext.ap().rearrange("(it p) c -> p it c", p=P)[
                                :, it, :
                            ],
                            st["acc"][:, :dv],
                        )

                order = [g * IT + i2 for g in range(NCH) for i2 in range(IT)]
                LAG = min(28, max(1, len(order) - 1))
                LAG0 = 3  # shallow lag inside ch0 so it completes in-AG
                anchor = ma.tile([1, 64], U16, tag="anchor", name="anchor",
                                 bufs=2)

                # NOTE: Tile rotates hardware DMAs over 8 completion
                # semaphores in EMISSION order; a DMA must wait for its
                # lane's previous occupant. AG-gated assembly DMAs must
                # therefore be emitted AFTER all of ch0's transposes, or
                # ch0 (which is AG-independent) transitively waits on the
                # collective.
                TRPE = 2 * IT  # chunks with PE transposes (AG overlap)
                sc_pend = collections.deque()  # (k, p_c) awaiting PE tr
                for k in order:
                    if k == IT:
                        # ch0 fully emitted; flush its transposes + PVs
                        # ahead of the first AG1-dependent scores (ch1
                        # PVs stay queued: they need the V AllGather)
                        while sc_pend:
                            kk, pc_t = sc_pend.popleft()
                            pend.append((kk, do_tr_pe(pc_t)))
                        while pend:
                            kk, pt = pend.popleft()
                            do_pv(kk, pt)
                        # kt assembly first (scores ch1 needs slot 1
                        # before PV needs V). anchor = static-offset
                        # pack_ag read carrying the collective wait for
                        # the dynamic-offset DMAs, which Tile can't
                        # track. tile_wait_until keeps the scheduler from
                        # hoisting these AG-gated DMAs ahead of ch0's
                        # transposes/exps in the engine streams (which
                        # parks those engines on the collective).
                        # everything on the SYNC engine: it is idle at
                        # AG1-completion (exps/evictions keep scalar busy
                        # for ~30us more), so assembly starts immediately
                        with tc.tile_wait_until(1.0):
                            nc.sync.dma_start(
                                anchor[:1, :], pack_k_ag[:][1:2, 0:64]
                            )
                            rk_sync = nc.sync.cc_rank(groups)
                            for s in range(1, NCH):
                                assemble_kt(s, rk_sync)
                            nc.sync.dma_start(
                                anchor[:1, :], pack_v_ag[:][0:1, 0:64]
                            )
                            for s in range(1, NCH):
                                assemble_v(s, rk_sync)
                    if k == TRPE:
                        # last PE-transposed chunk flushes before the
                        # first xbar-transposed one
                        while sc_pend:
                            kk, pc_t = sc_pend.popleft()
                            pend.append((kk, do_tr_pe(pc_t)))
                    res = do_scores(k)
                    if k < TRPE:
                        # PE-transpose lags scores by one chunk so the
                        # in-order PE never stalls on the exp chain
                        sc_pend.append((k, res))
                        if len(sc_pend) > 1:
                            kk, pc_t = sc_pend.popleft()
                            pend.append((kk, do_tr_pe(pc_t)))
                    else:
                        pend.append((k, res))
                    lag = LAG0 if k < IT else LAG
                    if len(pend) > lag:
                        kk, pt = pend.popleft()
                        do_pv(kk, pt)
                    # once every PV's V-slot is safely assembled (AG2 done
                    # well before ch4), amortize the deep lag down so the
                    # final drain isn't ~LAG serial PVs on the PE
                    if k >= 5 * IT and len(pend) > 10:
                        kk, pt = pend.popleft()
                        do_pv(kk, pt)
                while pend:
                    kk, pt = pend.popleft()
                    do_pv(kk, pt)

            phb_cm.__exit__(None, None, None)

    nc.finalize()
    return nc


_NC_CACHE = {}


def _get_nc(key):
    if key not in _NC_CACHE:
        n, d, dqk, dv, ncores = key
        _NC_CACHE[key] = build(n=n, d=d, dqk=dqk, dv=dv, ncores=ncores)
    return _NC_CACHE[key]


def run(x, Wq, Wk, Wv, trace=False):
    n, d = x.shape
    dqk = Wq.shape[0]
    dv = Wv.shape[0]
    ncores = N_CORES
    nsh = n // ncores
    nc = _get_nc((n, d, dqk, dv, ncores))

    x = np.ascontiguousarray(x, dtype=np.float32)
    Wq = np.ascontiguousarray(Wq, dtype=np.float32)
    Wk = np.ascontiguousarray(Wk, dtype=np.float32)
    Wv = np.ascontiguousarray(Wv, dtype=np.float32)

    in_maps = [
        {"x": x[r * nsh:(r + 1) * nsh], "Wq": Wq, "Wk": Wk, "Wv": Wv}
        for r in range(ncores)
    ]
    res = run_bass_kernel_spmd(
        nc, in_maps, core_ids=list(range(ncores)), trace=trace
    )
    out = np.concatenate([res.results[r]["out"] for r in range(ncores)], axis=0)
    return out, res


def kernel(x, Wq, Wk, Wv):
    out, _ = run(x, Wq, Wk, Wv)
    return out



# revision 6
# speedup vs baseline: 1.0315x; 1.0315x over previous
"""Distributed attention kernel for TRN2 (8 NeuronCores), v2.

Computes: softmax(sqrt(Dqk) * (x@Wq.T) @ (x@Wk.T).T) @ (x@Wv.T)
for x [8192, 1024], Wq/Wk/Wv [256, 1024], out [8192, 256].

Sharding: rows of x across 8 cores (sequence parallel), weights
replicated; K^T/V AllGathered; flash-style attention over each core's
1024 Q rows.

v2 changes over the 286-292us v1 baseline (see kernel_v1_baseline.py
for the v1 rationale docstring; all of it still applies):
  - warmup: a tiny dummy AllGather fires at kernel start. The first
    real collective in v1 paid a ~58us BARRIER (launch skew across the
    8 PJRT dispatches + CC-stream bootstrap); the dummy absorbs it
    concurrently with Phase A compute.
  - Phase A in f16: x is cast f32->f16 once per i-tile, all x^T/W^T
    PE transposes run in f16 (packed 4-to-a-PSUM-tile, single eviction
    per group), and the q/k/v projections are f16 (full PE rate vs
    2-pass f32r). CPU sim: adds ~1.5e-3 rel err (4.6e-3 -> ~7e-3 on
    HW, gate 2e-2). Cuts Phase A PE time roughly in half, so pack_k
    staging (which gates AG1) lands earlier.
  - q-projection moved BEFORE v-projection: ch0 scores only need q,
    and AG2 serializes behind AG1 on the CC stream anyway, so v can
    stage late without delaying AG2.
  - 4KB-packet assembly: pack_k is [P, CT*nsh] (per-partition 4KB
    contiguous rows), kt_full is [P, NCH-1, CT, nsh]; pack_v carries
    the ones-column in the payload (dvp=257 rows) so v_s slot writes
    are 4112B-contiguous. v1's 2KB/512B packets ran the slot-1
    assembly at ~50GB/s (10.3us!) which stalled ch1 scores.
  - xbar P^T transposes alternate sync/gpsimd engines (each costs
    ~1.3us of issuing-engine time; sync also carries assembly + out).
"""

import numpy as np

import concourse.bacc as bacc
import concourse.bass as bass
import concourse.mybir as mybir
import concourse.tile as tile
from concourse.bass_utils import run_bass_kernel_spmd
from concourse.masks import make_identity

F32 = mybir.dt.float32
BF16 = mybir.dt.bfloat16
F16 = mybir.dt.float16
U16 = mybir.dt.uint16

N_CORES = 8
N, D, DQK, DV = 8192, 1024, 256, 256
P = 128
CHUNK = 1024  # scores chunk width (2 PSUM banks)


def build(n=N, d=D, dqk=DQK, dv=DV, ncores=N_CORES):
    nsh = n // ncores
    IT = nsh // P       # i-tiles per core
    KT = d // P         # contract tiles for projections
    CT = dqk // P       # dqk tiles
    chunk = min(CHUNK, nsh)
    assert chunk == nsh, "rotation requires one chunk per rank"
    NCH = n // chunk    # score chunks per row == ncores
    W512 = min(512, chunk)
    NN = chunk // W512  # W512-wide matmuls per chunk
    scale = float(np.sqrt(dqk))
    dvp = dv + 1  # ones-column rides along: PV accumulates the softmax
    # denominator with all gamma corrections for free

    nc = bacc.Bacc(None, target_bir_lowering=False, num_devices=ncores)

    x_ext = nc.declare_dram_parameter("x", [nsh, d], F32, isOutput=False)
    wq_ext = nc.declare_dram_parameter("Wq", [dqk, d], F32, isOutput=False)
    wk_ext = nc.declare_dram_parameter("Wk", [dqk, d], F32, isOutput=False)
    wv_ext = nc.declare_dram_parameter("Wv", [dv, d], F32, isOutput=False)
    out_ext = nc.declare_dram_parameter("out", [nsh, dv], F32, isOutput=True)

    groups = [list(range(ncores))]

    with tile.TileContext(nc) as tc:
        with (
            tc.tile_pool(name="persist", bufs=1) as pp,
            tc.tile_pool(name="dramp", bufs=1, space="DRAM") as dp,
        ):
            # warmup collective: absorbs launch-skew / CC bootstrap so
            # the real AG1 doesn't pay a long barrier
            warm = dp.tile([1, 64], U16, name="warm")
            warm_ag = dp.tile([ncores, 64], U16, addr_space="Shared",
                              name="warm_ag")
            nc.gpsimd.collective_compute(
                "AllGather",
                mybir.AluOpType.bypass,
                replica_groups=groups,
                ins=[warm[:].opt()],
                outs=[warm_ag[:].opt()],
            )

            # Packed AG buffers. Per-partition rows are 4KB-contiguous
            # so the assembly DMAs move 4KB packets:
            #   pack_k: partition p holds k^T rows (p, 128+p) = CT*nsh
            #           f16 = 4KB — AllGathered FIRST (gates scores)
            #   pack_v: v as [IT, dvp] bf16 slabs (ones-column packed
            #           in!) split over rows 2p, 2p+1 — gathered second
            pack_k = dp.tile([P, CT * nsh], U16, name="pack_k")
            pack_k_ag = dp.tile(
                [ncores * P, CT * nsh], U16, addr_space="Shared",
                name="pack_k_ag",
            )
            VROW = IT * dvp // 2  # 1028 u16 per packed v row
            pack_v = dp.tile([2 * P, VROW], U16, name="pack_v")
            pack_v_ag = dp.tile(
                [ncores * 2 * P, VROW], U16, addr_space="Shared",
                name="pack_v_ag",
            )

            qt_s = pp.tile([P, CT, nsh], F16, tag="qt")
            kt_loc = pp.tile([P, CT, nsh], F16, tag="ktloc")
            v_loc = pp.tile([P, IT, dvp], BF16, tag="vloc")
            nc.vector.memset(v_loc[:, :, dv:dvp], 1.0)
            ident_h = pp.tile([P, P], F16, tag="identh")
            make_identity(nc, ident_h[:])
            ident_b = pp.tile([P, P], BF16, tag="identb")
            make_identity(nc, ident_b[:])

            # ================= Phase A =================
            with (
                tc.tile_pool(name="phA", bufs=1) as pa,
                tc.tile_pool(name="phA_psum", bufs=1, space="PSUM") as paps,
            ):
                x_nat = pa.tile([P, IT, d], F32, tag="xnat")
                xh = pa.tile([P, IT, d], F16, tag="xh")
                w_nat = pa.tile([P, 3 * CT, d], F32, tag="wnat")
                w_exts = {0: wq_ext, 1: wk_ext, 2: wv_ext}

                def w_dma(wi):
                    nc.scalar.dma_start(
                        w_nat[:, wi * CT:(wi + 1) * CT, :],
                        w_exts[wi].ap().rearrange("(ct p) d -> p ct d", p=P),
                    )

                def x_dma(it, eng):
                    eng.dma_start(
                        x_nat[:, it, :],
                        x_ext.ap().rearrange("(it p) d -> p it d", p=P)[:, it, :],
                    )

                # scalar queue: Wk FIRST (it gates the Wk transposes ->
                # k-proj -> AG1 path); x evens on sync in parallel
                w_dma(1)
                for it in range(0, IT, 2):
                    x_dma(it, nc.sync)
                for it in range(1, IT, 2):
                    x_dma(it, nc.scalar)
                w_dma(0)
                w_dma(2)

                # cast x to f16 as tiles land (transposes + projections
                # run in f16 — halves PE cycles vs f32)
                for it in range(IT):
                    if it % 2:
                        nc.scalar.copy(xh[:, it, :], x_nat[:, it, :])
                    else:
                        nc.vector.tensor_copy(xh[:, it, :], x_nat[:, it, :])

                xt_s = pa.tile([P, KT, nsh], F16, tag="xt")
                wt_s = pa.tile([P, 3 * KT, dqk], F16, tag="wt")

                ei = 0  # eviction engine alternator

                def evict(dst, src):
                    nonlocal ei
                    ei += 1
                    if ei % 2:
                        nc.vector.tensor_copy(dst, src)
                    else:
                        nc.scalar.copy(dst, src)

                ich_w = min(512, nsh)

                # f16 transposes packed 4-per-PSUM-tile with one
                # strided eviction per group (vs v1's per-tile evicts)
                def x_transposes(it_range):
                    for it in it_range:
                        for g in range(KT // 4):
                            tp = paps.tile([P, 256], F32, tag="tp", bufs=4)
                            tph = tp[:].bitcast(F16)  # [P, 512]
                            for j in range(4):
                                kt = 4 * g + j
                                nc.tensor.transpose(
                                    tph[:, j * P:(j + 1) * P],
                                    xh[:, it, kt * P:(kt + 1) * P],
                                    ident_h[:],
                                )
                            evict(
                                xt_s[:, 4 * g:4 * g + 4, it * P:(it + 1) * P],
                                tph.rearrange("p (f i) -> p f i", f=4),
                            )

                def w_transposes(wi):
                    # cast this W to f16 in-place groups then transpose:
                    # pack (kt pair x ct pair) -> contiguous 512 dst
                    wh = pa.tile([P, CT, d], F16, tag=f"wh{wi}")
                    nc.vector.tensor_copy(wh[:, :, :], w_nat[:, wi * CT:(wi + 1) * CT, :])
                    for g in range(KT // 2):
                        tp = paps.tile([P, 256], F32, tag="tp", bufs=4)
                        tph = tp[:].bitcast(F16)  # [P, 512]
                        for j in range(2):
                            kt = 2 * g + j
                            for ct in range(CT):
                                nc.tensor.transpose(
                                    tph[:, (j * CT + ct) * P:(j * CT + ct + 1) * P],
                                    wh[:, ct, kt * P:(kt + 1) * P],
                                    ident_h[:],
                                )
                        evict(
                            wt_s[:, wi * KT + 2 * g:wi * KT + 2 * g + 2, :],
                            tph.rearrange("p (f i) -> p f i", f=2),
                        )

                def k_proj(ich):
                    for ct in range(CT):
                        psk = paps.tile([P, ich_w], F32, tag="pqk", bufs=2)
                        for kt in range(KT):
                            nc.tensor.matmul(
                                psk[:],
                                wt_s[:, 1 * KT + kt, ct * P:(ct + 1) * P],
                                xt_s[:, kt, ich * ich_w:(ich + 1) * ich_w],
                                start=(kt == 0),
                                stop=(kt == KT - 1),
                            )
                        nc.vector.tensor_copy(
                            kt_loc[:, ct, ich * ich_w:(ich + 1) * ich_w],
                            psk[:],
                        )
                    # stage this ich half immediately (AG waits on staging)
                    nc.sync.dma_start(
                        pack_k[:].bitcast(F16).rearrange(
                            "p (ct i) -> p ct i", ct=CT
                        )[:, :, ich * ich_w:(ich + 1) * ich_w],
                        kt_loc[:, :, ich * ich_w:(ich + 1) * ich_w],
                    )

                # k_proj(ich) only needs x^T of i-rows in that half:
                # interleave so AG1 triggers as early as possible
                ipw = ich_w // P
                x_transposes(range(ipw))
                w_transposes(1)           # Wk
                k_proj(0)
                x_transposes(range(ipw, IT))
                k_proj(1)
                # K AllGather fires as soon as k^T is staged; scores for
                # ch>=1 depend only on this one
                nc.gpsimd.collective_compute(
                    "AllGather",
                    mybir.AluOpType.bypass,
                    replica_groups=groups,
                    ins=[pack_k[:].opt()],
                    outs=[pack_k_ag[:].opt()],
                )

                # Wq transposes + q^T projection next: ch0 scores only
                # need q, and AG2 serializes behind AG1 on the CC
                # stream anyway so v can stage later without cost.
                # ich outer so q rows 0-511 finish first.
                w_transposes(0)
                for ich in range(nsh // ich_w):
                    for ct in range(CT):
                        psq = paps.tile([P, ich_w], F32, tag="pqk", bufs=2)
                        for kt in range(KT):
                            nc.tensor.matmul(
                                psq[:],
                                wt_s[:, 0 * KT + kt, ct * P:(ct + 1) * P],
                                xt_s[:, kt, ich * ich_w:(ich + 1) * ich_w],
                                start=(kt == 0),
                                stop=(kt == KT - 1),
                            )
                        nc.vector.tensor_scalar_mul(
                            qt_s[:, ct, ich * ich_w:(ich + 1) * ich_w],
                            psq[:],
                            scale,
                        )

                w_transposes(2)           # Wv
                # v projections
                for it in range(IT):
                    psv = paps.tile([P, dv], F32, tag="psv", bufs=2)
                    for kt in range(KT):
                        nc.tensor.matmul(
                            psv[:],
                            xt_s[:, kt, it * P:(it + 1) * P],
                            wt_s[:, 2 * KT + kt, :dqk],
                            start=(kt == 0),
                            stop=(kt == KT - 1),
                        )
                    nc.vector.tensor_copy(v_loc[:, it, :dv], psv[:])
                # v packed WITH the ones column: partition p's [IT, dvp]
                # slab -> rows 2p, 2p+1 (so receiver slot writes are one
                # contiguous 4112B run per partition)
                nc.sync.dma_start(
                    pack_v[:].bitcast(BF16).rearrange(
                        "(p r) i -> p (r i)", p=P
                    ),
                    v_loc[:, :, :],
                )
                # V AllGather (runs after the K one on the CC stream)
                nc.gpsimd.collective_compute(
                    "AllGather",
                    mybir.AluOpType.bypass,
                    replica_groups=groups,
                    ins=[pack_v[:].opt()],
                    outs=[pack_v_ag[:].opt()],
                )

            phb_cm = tc.tile_pool(name="phB", bufs=1)
            phb = phb_cm.__enter__()
            # slots 1..NCH-1 only (slot 0 is local kt_loc / v_loc)
            kt_full = phb.tile([P, NCH - 1, CT, nsh], F16, tag="ktf",
                               name="kt_full")
            v_s = phb.tile([P, (NCH - 1) * IT, dvp], BF16, tag="vs",
                           name="v_s")

            pag_h = pack_k_ag[:].bitcast(F16)
            pag_b = pack_v_ag[:].bitcast(BF16)

            def assemble_v(s, rk_sync):
                # slot s holds rank (my_rank + s) % ncores
                vrow = ((rk_sync + s) % ncores) * 2 * P
                nc.sync.dma_start(
                    v_s[:, (s - 1) * IT:s * IT, :],
                    pag_b[bass.ds(vrow, 2 * P), :].rearrange(
                        "(p r) i -> p (r i)", p=P
                    ),
                )

            def assemble_kt(s, rk_kt):
                row = ((rk_kt + s) % ncores) * P
                nc.sync.dma_start(
                    kt_full[:, s - 1, :, :],
                    pag_h[bass.ds(row, P), :].rearrange(
                        "p (ct i) -> p ct i", ct=CT
                    ),
                )

            # ================= Main attention loop =================
            # Streaming flash-attention: each 1024-wide chunk flows
            # MM -> row-max(DVE) -> exp(ACT, running-max bias) -> P^T
            # transpose -> PV -> one-op accumulator merge.
            with (
                tc.tile_pool(name="mainA", bufs=8) as ma,
                tc.tile_pool(name="chunkp", bufs=2) as cp,
                tc.tile_pool(name="scores_psum", bufs=3, space="PSUM") as sps,
                tc.tile_pool(name="out_psum", bufs=2, space="PSUM") as ops,
            ):
                JPC = chunk // P  # j-tiles per chunk (== IT)

                NTOT = IT * NCH
                stats = []
                for it in range(IT):
                    st = {
                        "mneg": ma.tile([P, NCH], F32, tag="mneg", name="mneg"),
                        "nmr": ma.tile([P, NCH], F32, tag="nmr", name="nmr"),
                        "rinv": ma.tile([P, 1], F32, tag="rinv", name="rinv"),
                        "gam": ma.tile([P, NCH], F32, tag="gam", name="gam"),
                        "acc": ma.tile([P, dvp], F32, tag="acc", name="acc"),
                    }
                    stats.append(st)

                import collections
                pend = collections.deque()  # (k, pt_c) with deep PV lag
                tri = 0  # xbar transpose engine alternator

                def do_scores(k):
                    nonlocal tri
                    ch, it = divmod(k, IT)
                    st = stats[it]
                    ps = sps.tile([P, chunk], F32, tag="s", name="ps")
                    for ct in range(CT):
                        for nn in range(NN):
                            nc.tensor.matmul(
                                ps[:, nn * W512:(nn + 1) * W512],
                                qt_s[:, ct, it * P:(it + 1) * P],
                                (kt_loc[:, ct, nn * W512:(nn + 1) * W512]
                                 if ch == 0 else
                                 kt_full[
                                     :, ch - 1, ct,
                                     nn * W512:(nn + 1) * W512,
                                 ]),
                                start=(ct == 0),
                                stop=(ct == CT - 1),
                                skip_group_check=True,
                            )
                    nc.vector.reduce_max(
                        st["mneg"][:, ch:ch + 1], ps[:],
                        axis=mybir.AxisListType.X, negate=True,
                    )
                    if ch > 0:
                        nc.vector.tensor_tensor(
                            st["nmr"][:, ch:ch + 1], st["nmr"][:, ch - 1:ch],
                            st["mneg"][:, ch:ch + 1], op=mybir.AluOpType.min,
                        )
                    else:
                        nc.vector.tensor_copy(st["nmr"][:, :1], st["mneg"][:, :1])
                    p_c = cp.tile([P, chunk], BF16, tag="p", name="p_c", bufs=16)
                    # bias = running max -> P is exp(s - m_run(ch)); no
                    # beta, and no accum_out: the denominator comes from
                    # the PV ones-column
                    nc.scalar.activation(
                        p_c[:], ps[:],
                        mybir.ActivationFunctionType.Exp,
                        bias=st["nmr"][:, ch:ch + 1],
                        scale=1.0,
                    )
                    if ch < 2:
                        # xbar transposes are serialized against in-flight
                        # collectives; ch0/ch1 (which overlap the K and V
                        # AllGathers) must transpose on the PE instead
                        return p_c
                    pt_c = cp.tile([P, JPC, P], BF16, tag="pt", name="pt_c", bufs=30)
                    # DMA_TRANSPOSE burns ~1.3us of ISSUING-ENGINE time;
                    # keep them all on sync (only SP/Activation are
                    # HWDGE-capable, and scalar is busy with exps).
                    # ch2/ch3's transposes queue behind the v-anchor in
                    # sync's stream, which keeps the xbar (hardware-
                    # serialized against in-flight collectives) away
                    # from the V AllGather.
                    nc.sync.dma_start_transpose(pt_c[:], p_c[:])
                    return pt_c

                def do_tr_pe(p_c_t):
                    # P^T via PE for ch0/ch1 (PSUM is full: borrow a
                    # scores-ring tile, pack the 8 bf16 transpose outputs
                    # into its first half via bitcast slices)
                    pt_c = cp.tile([P, JPC, P], BF16, tag="pt", name="pt_c", bufs=30)
                    ps_tr = sps.tile([P, chunk], F32, tag="s", name="ps_tr")
                    for j2 in range(JPC):
                        tpp = ps_tr[:, j2 * 64:(j2 + 1) * 64].bitcast(BF16)
                        nc.tensor.transpose(
                            tpp, p_c_t[:, j2 * P:(j2 + 1) * P], ident_b[:]
                        )
                        if j2 % 2:
                            nc.vector.tensor_copy(pt_c[:, j2, :], tpp)
                        else:
                            nc.scalar.copy(pt_c[:, j2, :], tpp)
                    return pt_c

                def do_pv(k, pt_c):
                    ch, it = divmod(k, IT)
                    st = stats[it]
                    po = ops.tile([P, dvp], F32, tag="po", name="po")
                    for j2 in range(JPC):
                        nc.tensor.matmul(
                            po[:], pt_c[:, j2, :],
                            (v_loc[:, j2, :] if ch == 0 else
                             v_s[:, (ch - 1) * JPC + j2, :]),
                            start=(j2 == 0), stop=(j2 == JPC - 1),
                        )
                    if ch == 0:
                        nc.vector.tensor_copy(st["acc"][:], po[:])
                    else:
                        # gamma = exp(m_run(ch-1) - m_run(ch))
                        nc.scalar.activation(
                            st["gam"][:, ch:ch + 1], st["nmr"][:, ch - 1:ch],
                            mybir.ActivationFunctionType.Exp,
                            bias=st["nmr"][:, ch:ch + 1], scale=-1.0,
                        )
                        # acc = acc*gamma + po (P already carries beta via
                        # the running-max exp bias)
                        nc.vector.scalar_tensor_tensor(
                            st["acc"][:], st["acc"][:], st["gam"][:, ch:ch + 1],
                            po[:],
                            op0=mybir.AluOpType.mult,
                            op1=mybir.AluOpType.add,
                        )
                    if ch == NCH - 1:
                        # denominator rode along in the ones-column
                        nc.vector.reciprocal(
                            st["rinv"][:], st["acc"][:, dv:dvp]
                        )
                        nc.vector.tensor_scalar_mul(
                            st["acc"][:, :dv], st["acc"][:, :dv], st["rinv"][:]
                        )
                        nc.sync.dma_start(
                            out_ext.ap().rearrange("(it p) c -> p it c", p=P)[
                                :, it, :
                            ],
                            st["acc"][:, :dv],
                        )

                order = [g * IT + i2 for g in range(NCH) for i2 in range(IT)]
                LAG = min(28, max(1, len(order) - 1))
                LAG0 = 3  # shallow lag inside ch0 so it completes in-AG
                anchor = ma.tile([1, 64], U16, tag="anchor", name="anchor",
                                 bufs=2)

                # NOTE: Tile rotates hardware DMAs over 8 completion
                # semaphores in EMISSION order; a DMA must wait for its
                # lane's previous occupant. AG-gated assembly DMAs must
                # therefore be emitted AFTER all of ch0's transposes, or
                # ch0 (which is AG-independent) transitively waits on the
                # collective.
                TRPE = 2 * IT  # chunks with PE transposes (AG overlap)
                sc_pend = collections.deque()  # (k, p_c) awaiting PE tr
                for k in order:
                    if k == IT:
                        # ch0 fully emitted; flush its transposes + PVs
                        # ahead of the first AG1-dependent scores (ch1
                        # PVs stay queued: they need the V AllGather)
                        while sc_pend:
                            kk, pc_t = sc_pend.popleft()
                            pend.append((kk, do_tr_pe(pc_t)))
                        while pend:
                            kk, pt = pend.popleft()
                            do_pv(kk, pt)
                        # kt assembly first (scores ch1 needs slot 1
                        # before PV needs V). anchor = static-offset
                        # pack_ag read carrying the collective wait for
                        # the dynamic-offset DMAs, which Tile can't
                        # track. tile_wait_until keeps the scheduler from
                        # hoisting these AG-gated DMAs ahead of ch0's
                        # transposes/exps in the engine streams (which
                        # parks those engines on the collective).
                        # everything on the SYNC engine: it is idle at
                        # AG1-completion, so assembly starts immediately
                        with tc.tile_wait_until(1.0):
                            nc.sync.dma_start(
                                anchor[:1, :], pack_k_ag[:][1:2, 0:64]
                            )
                            rk_sync = nc.sync.cc_rank(groups)
                            for s in range(1, NCH):
                                assemble_kt(s, rk_sync)
                            nc.sync.dma_start(
                                anchor[:1, :], pack_v_ag[:][0:1, 0:64]
                            )
                            for s in range(1, NCH):
                                assemble_v(s, rk_sync)
                    if k == TRPE:
                        # last PE-transposed chunk flushes before the
                        # first xbar-transposed one
                        while sc_pend:
                            kk, pc_t = sc_pend.popleft()
                            pend.append((kk, do_tr_pe(pc_t)))
                    res = do_scores(k)
                    if k < TRPE:
                        # PE-transpose lags scores by one chunk so the
                        # in-order PE never stalls on the exp chain
                        sc_pend.append((k, res))
                        if len(sc_pend) > 1:
                            kk, pc_t = sc_pend.popleft()
                            pend.append((kk, do_tr_pe(pc_t)))
                    else:
                        pend.append((k, res))
                    lag = LAG0 if k < IT else LAG
                    if len(pend) > lag:
                        kk, pt = pend.popleft()
                        do_pv(kk, pt)
                    # once every PV's V-slot is safely assembled (AG2 done
                    # well before ch4), amortize the deep lag down so the
                    # final drain isn't ~LAG serial PVs on the PE
                    if k >= 5 * IT and len(pend) > 10:
                        kk, pt = pend.popleft()
                        do_pv(kk, pt)
                while pend:
                    kk, pt = pend.popleft()
                    do_pv(kk, pt)

            phb_cm.__exit__(None, None, None)

    nc.finalize()
    return nc


_NC_CACHE = {}


def _get_nc(key):
    if key not in _NC_CACHE:
        n, d, dqk, dv, ncores = key
        _NC_CACHE[key] = build(n=n, d=d, dqk=dqk, dv=dv, ncores=ncores)
    return _NC_CACHE[key]


def run(x, Wq, Wk, Wv, trace=False):
    n, d = x.shape
    dqk = Wq.shape[0]
    dv = Wv.shape[0]
    ncores = N_CORES
    nsh = n // ncores
    nc = _get_nc((n, d, dqk, dv, ncores))

    x = np.ascontiguousarray(x, dtype=np.float32)
    Wq = np.ascontiguousarray(Wq, dtype=np.float32)
    Wk = np.ascontiguousarray(Wk, dtype=np.float32)
    Wv = np.ascontiguousarray(Wv, dtype=np.float32)

    in_maps = [
        {"x": x[r * nsh:(r + 1) * nsh], "Wq": Wq, "Wk": Wk, "Wv": Wv}
        for r in range(ncores)
    ]
    res = run_bass_kernel_spmd(
        nc, in_maps, core_ids=list(range(ncores)), trace=trace
    )
    out = np.concatenate([res.results[r]["out"] for r in range(ncores)], axis=0)
    return out, res


def kernel(x, Wq, Wk, Wv):
    out, _ = run(x, Wq, Wk, Wv)
    return out


# revision 10
# speedup vs baseline: 1.0317x; 1.0002x over previous
"""Distributed attention kernel for TRN2 (8 NeuronCores), v2.

Computes: softmax(sqrt(Dqk) * (x@Wq.T) @ (x@Wk.T).T) @ (x@Wv.T)
for x [8192, 1024], Wq/Wk/Wv [256, 1024], out [8192, 256].

Sharding: rows of x across 8 cores (sequence parallel), weights
replicated; K^T/V AllGathered; flash-style attention over each core's
1024 Q rows.

v2 changes over the 286-292us v1 baseline (see kernel_v1_baseline.py
for the v1 rationale docstring; all of it still applies):
  - warmup: a tiny dummy AllGather fires at kernel start. The first
    real collective in v1 paid a ~58us BARRIER (launch skew across the
    8 PJRT dispatches + CC-stream bootstrap); the dummy absorbs it
    concurrently with Phase A compute.
  - Phase A in f16: x is cast f32->f16 once per i-tile, all x^T/W^T
    PE transposes run in f16 (packed 4-to-a-PSUM-tile, single eviction
    per group), and the q/k/v projections are f16 (full PE rate vs
    2-pass f32r). CPU sim: adds ~1.5e-3 rel err (4.6e-3 -> ~7e-3 on
    HW, gate 2e-2). Cuts Phase A PE time roughly in half, so pack_k
    staging (which gates AG1) lands earlier.
  - q-projection moved BEFORE v-projection: ch0 scores only need q,
    and AG2 serializes behind AG1 on the CC stream anyway, so v can
    stage late without delaying AG2.
  - 4KB-packet assembly: pack_k is [P, CT*nsh] (per-partition 4KB
    contiguous rows), kt_full is [P, NCH-1, CT, nsh]; pack_v carries
    the ones-column in the payload (dvp=257 rows) so v_s slot writes
    are 4112B-contiguous. v1's 2KB/512B packets ran the slot-1
    assembly at ~50GB/s (10.3us!) which stalled ch1 scores.
  - xbar P^T transposes alternate sync/gpsimd engines (each costs
    ~1.3us of issuing-engine time; sync also carries assembly + out).
"""

import numpy as np

import concourse.bacc as bacc
import concourse.bass as bass
import concourse.mybir as mybir
import concourse.tile as tile
from concourse.bass_utils import run_bass_kernel_spmd
from concourse.masks import make_identity

F32 = mybir.dt.float32
BF16 = mybir.dt.bfloat16
F16 = mybir.dt.float16
U16 = mybir.dt.uint16

N_CORES = 8
N, D, DQK, DV = 8192, 1024, 256, 256
P = 128
CHUNK = 1024  # scores chunk width (2 PSUM banks)


def build(n=N, d=D, dqk=DQK, dv=DV, ncores=N_CORES):
    nsh = n // ncores
    IT = nsh // P       # i-tiles per core
    KT = d // P         # contract tiles for projections
    CT = dqk // P       # dqk tiles
    chunk = min(CHUNK, nsh)
    assert chunk == nsh, "rotation requires one chunk per rank"
    NCH = n // chunk    # score chunks per row == ncores
    W512 = min(512, chunk)
    NN = chunk // W512  # W512-wide matmuls per chunk
    scale = float(np.sqrt(dqk))
    dvp = dv + 1  # ones-column rides along: PV accumulates the softmax
    # denominator with all gamma corrections for free

    nc = bacc.Bacc(None, target_bir_lowering=False, num_devices=ncores)

    x_ext = nc.declare_dram_parameter("x", [nsh, d], F32, isOutput=False)
    wq_ext = nc.declare_dram_parameter("Wq", [dqk, d], F32, isOutput=False)
    wk_ext = nc.declare_dram_parameter("Wk", [dqk, d], F32, isOutput=False)
    wv_ext = nc.declare_dram_parameter("Wv", [dv, d], F32, isOutput=False)
    out_ext = nc.declare_dram_parameter("out", [nsh, dv], F32, isOutput=True)

    groups = [list(range(ncores))]

    with tile.TileContext(nc) as tc:
        with (
            tc.tile_pool(name="persist", bufs=1) as pp,
            tc.tile_pool(name="dramp", bufs=1, space="DRAM") as dp,
        ):
            # warmup collective: absorbs launch-skew / CC bootstrap so
            # the real AG1 doesn't pay a long barrier
            warm = dp.tile([1, 64], U16, name="warm")
            warm_ag = dp.tile([ncores, 64], U16, addr_space="Shared",
                              name="warm_ag")
            nc.gpsimd.collective_compute(
                "AllGather",
                mybir.AluOpType.bypass,
                replica_groups=groups,
                ins=[warm[:].opt()],
                outs=[warm_ag[:].opt()],
            )

            # Packed AG buffers. Per-partition rows are 4KB-contiguous
            # so the assembly DMAs move 4KB packets:
            #   pack_k: partition p holds k^T rows (p, 128+p) = CT*nsh
            #           f16 = 4KB — AllGathered FIRST (gates scores)
            #   pack_v: v as [IT, dvp] bf16 slabs (ones-column packed
            #           in!) split over rows 2p, 2p+1 — gathered second
            pack_k = dp.tile([P, CT * nsh], U16, name="pack_k")
            pack_k_ag = dp.tile(
                [ncores * P, CT * nsh], U16, addr_space="Shared",
                name="pack_k_ag",
            )
            VROW = IT * dvp // 2  # 1028 u16 per packed v row
            pack_v = dp.tile([2 * P, VROW], U16, name="pack_v")
            pack_v_ag = dp.tile(
                [ncores * 2 * P, VROW], U16, addr_space="Shared",
                name="pack_v_ag",
            )

            qt_s = pp.tile([P, CT, nsh], F16, tag="qt")
            kt_loc = pp.tile([P, CT, nsh], F16, tag="ktloc")
            v_loc = pp.tile([P, IT, dvp], BF16, tag="vloc")
            nc.vector.memset(v_loc[:, :, dv:dvp], 1.0)
            ident_h = pp.tile([P, P], F16, tag="identh")
            make_identity(nc, ident_h[:])
            ident_b = pp.tile([P, P], BF16, tag="identb")
            make_identity(nc, ident_b[:])

            # ================= Phase A =================
            with (
                tc.tile_pool(name="phA", bufs=1) as pa,
                tc.tile_pool(name="phA_psum", bufs=1, space="PSUM") as paps,
            ):
                x_nat = pa.tile([P, IT, d], F32, tag="xnat")
                xh = pa.tile([P, IT, d], F16, tag="xh")
                w_nat = pa.tile([P, 3 * CT, d], F32, tag="wnat")
                w_exts = {0: wq_ext, 1: wk_ext, 2: wv_ext}

                def w_dma(wi):
                    nc.scalar.dma_start(
                        w_nat[:, wi * CT:(wi + 1) * CT, :],
                        w_exts[wi].ap().rearrange("(ct p) d -> p ct d", p=P),
                    )

                def x_dma(it, eng):
                    eng.dma_start(
                        x_nat[:, it, :],
                        x_ext.ap().rearrange("(it p) d -> p it d", p=P)[:, it, :],
                    )

                # scalar queue: Wk FIRST (it gates the Wk transposes ->
                # k-proj -> AG1 path); x evens on sync in parallel
                w_dma(1)
                for it in range(0, IT, 2):
                    x_dma(it, nc.sync)
                for it in range(1, IT, 2):
                    x_dma(it, nc.scalar)
                w_dma(0)
                w_dma(2)

                # cast x to f16 as tiles land (transposes + projections
                # run in f16 — halves PE cycles vs f32)
                for it in range(IT):
                    if it % 2:
                        nc.scalar.copy(xh[:, it, :], x_nat[:, it, :])
                    else:
                        nc.vector.tensor_copy(xh[:, it, :], x_nat[:, it, :])

                xt_s = pa.tile([P, KT, nsh], F16, tag="xt")
                wt_s = pa.tile([P, 3 * KT, dqk], F16, tag="wt")

                ei = 0  # eviction engine alternator

                def evict(dst, src):
                    nonlocal ei
                    ei += 1
                    if ei % 2:
                        nc.vector.tensor_copy(dst, src)
                    else:
                        nc.scalar.copy(dst, src)

                ich_w = min(512, nsh)

                # f16 transposes packed 4-per-PSUM-tile with one
                # strided eviction per group (vs v1's per-tile evicts)
                def x_transposes(it_range):
                    for it in it_range:
                        for g in range(KT // 4):
                            tp = paps.tile([P, 256], F32, tag="tp", bufs=4)
                            tph = tp[:].bitcast(F16)  # [P, 512]
                            for j in range(4):
                                kt = 4 * g + j
                                nc.tensor.transpose(
                                    tph[:, j * P:(j + 1) * P],
                                    xh[:, it, kt * P:(kt + 1) * P],
                                    ident_h[:],
                                )
                            evict(
                                xt_s[:, 4 * g:4 * g + 4, it * P:(it + 1) * P],
                                tph.rearrange("p (f i) -> p f i", f=4),
                            )

                def w_transposes(wi):
                    # cast this W to f16 in-place groups then transpose:
                    # pack (kt pair x ct pair) -> contiguous 512 dst
                    wh = pa.tile([P, CT, d], F16, tag=f"wh{wi}")
                    nc.vector.tensor_copy(wh[:, :, :], w_nat[:, wi * CT:(wi + 1) * CT, :])
                    for g in range(KT // 2):
                        tp = paps.tile([P, 256], F32, tag="tp", bufs=4)
                        tph = tp[:].bitcast(F16)  # [P, 512]
                        for j in range(2):
                            kt = 2 * g + j
                            for ct in range(CT):
                                nc.tensor.transpose(
                                    tph[:, (j * CT + ct) * P:(j * CT + ct + 1) * P],
                                    wh[:, ct, kt * P:(kt + 1) * P],
                                    ident_h[:],
                                )
                        evict(
                            wt_s[:, wi * KT + 2 * g:wi * KT + 2 * g + 2, :],
                            tph.rearrange("p (f i) -> p f i", f=2),
                        )

                def k_proj(ich):
                    for ct in range(CT):
                        psk = paps.tile([P, ich_w], F32, tag="pqk", bufs=2)
                        for kt in range(KT):
                            nc.tensor.matmul(
                                psk[:],
                                wt_s[:, 1 * KT + kt, ct * P:(ct + 1) * P],
                                xt_s[:, kt, ich * ich_w:(ich + 1) * ich_w],
                                start=(kt == 0),
                                stop=(kt == KT - 1),
                            )
                        nc.vector.tensor_copy(
                            kt_loc[:, ct, ich * ich_w:(ich + 1) * ich_w],
                            psk[:],
                        )
                    # stage this ich half immediately (AG waits on staging)
                    nc.sync.dma_start(
                        pack_k[:].bitcast(F16).rearrange(
                            "p (ct i) -> p ct i", ct=CT
                        )[:, :, ich * ich_w:(ich + 1) * ich_w],
                        kt_loc[:, :, ich * ich_w:(ich + 1) * ich_w],
                    )

                # k_proj(ich) only needs x^T of i-rows in that half:
                # interleave so AG1 triggers as early as possible
                ipw = ich_w // P
                x_transposes(range(ipw))
                w_transposes(1)           # Wk
                k_proj(0)
                x_transposes(range(ipw, IT))
                k_proj(1)
                # K AllGather fires as soon as k^T is staged; scores for
                # ch>=1 depend only on this one
                nc.gpsimd.collective_compute(
                    "AllGather",
                    mybir.AluOpType.bypass,
                    replica_groups=groups,
                    ins=[pack_k[:].opt()],
                    outs=[pack_k_ag[:].opt()],
                )

                # Wq transposes + q^T projection next: ch0 scores only
                # need q, and AG2 serializes behind AG1 on the CC
                # stream anyway so v can stage later without cost.
                # ich outer so q rows 0-511 finish first.
                w_transposes(0)
                for ich in range(nsh // ich_w):
                    for ct in range(CT):
                        psq = paps.tile([P, ich_w], F32, tag="pqk", bufs=2)
                        for kt in range(KT):
                            nc.tensor.matmul(
                                psq[:],
                                wt_s[:, 0 * KT + kt, ct * P:(ct + 1) * P],
                                xt_s[:, kt, ich * ich_w:(ich + 1) * ich_w],
                                start=(kt == 0),
                                stop=(kt == KT - 1),
                            )
                        nc.vector.tensor_scalar_mul(
                            qt_s[:, ct, ich * ich_w:(ich + 1) * ich_w],
                            psq[:],
                            scale,
                        )

                w_transposes(2)           # Wv
                # v projections
                for it in range(IT):
                    psv = paps.tile([P, dv], F32, tag="psv", bufs=2)
                    for kt in range(KT):
                        nc.tensor.matmul(
                            psv[:],
                            xt_s[:, kt, it * P:(it + 1) * P],
                            wt_s[:, 2 * KT + kt, :dqk],
                            start=(kt == 0),
                            stop=(kt == KT - 1),
                        )
                    nc.vector.tensor_copy(v_loc[:, it, :dv], psv[:])
                # v packed WITH the ones column: partition p's [IT, dvp]
                # slab -> rows 2p, 2p+1 (so receiver slot writes are one
                # contiguous 4112B run per partition)
                nc.sync.dma_start(
                    pack_v[:].bitcast(BF16).rearrange(
                        "(p r) i -> p (r i)", p=P
                    ),
                    v_loc[:, :, :],
                )
                # V AllGather (runs after the K one on the CC stream)
                nc.gpsimd.collective_compute(
                    "AllGather",
                    mybir.AluOpType.bypass,
                    replica_groups=groups,
                    ins=[pack_v[:].opt()],
                    outs=[pack_v_ag[:].opt()],
                )

            phb_cm = tc.tile_pool(name="phB", bufs=1)
            phb = phb_cm.__enter__()
            # slots 1..NCH-1 only (slot 0 is local kt_loc / v_loc)
            kt_full = phb.tile([P, NCH - 1, CT, nsh], F16, tag="ktf",
                               name="kt_full")
            v_s = phb.tile([P, (NCH - 1) * IT, dvp], BF16, tag="vs",
                           name="v_s")

            pag_h = pack_k_ag[:].bitcast(F16)
            pag_b = pack_v_ag[:].bitcast(BF16)

            def assemble_v(s, rk):
                # slot s holds rank (my_rank + s) % ncores
                vrow = ((rk + s) % ncores) * 2 * P
                nc.sync.dma_start(
                    v_s[:, (s - 1) * IT:s * IT, :],
                    pag_b[bass.ds(vrow, 2 * P), :].rearrange(
                        "(p r) i -> p (r i)", p=P
                    ),
                )

            def assemble_kt(s, rk, eng):
                row = ((rk + s) % ncores) * P
                eng.dma_start(
                    kt_full[:, s - 1, :, :],
                    pag_h[bass.ds(row, P), :].rearrange(
                        "p (ct i) -> p ct i", ct=CT
                    ),
                )

            # ================= Main attention loop =================
            # Streaming flash-attention: each 1024-wide chunk flows
            # MM -> row-max(DVE) -> exp(ACT, running-max bias) -> P^T
            # transpose -> PV -> one-op accumulator merge.
            with (
                tc.tile_pool(name="mainA", bufs=8) as ma,
                tc.tile_pool(name="chunkp", bufs=2) as cp,
                tc.tile_pool(name="scores_psum", bufs=3, space="PSUM") as sps,
                tc.tile_pool(name="out_psum", bufs=2, space="PSUM") as ops,
            ):
                JPC = chunk // P  # j-tiles per chunk (== IT)

                NTOT = IT * NCH
                stats = []
                for it in range(IT):
                    st = {
                        "mneg": ma.tile([P, NCH], F32, tag="mneg", name="mneg"),
                        "nmr": ma.tile([P, NCH], F32, tag="nmr", name="nmr"),
                        "rinv": ma.tile([P, 1], F32, tag="rinv", name="rinv"),
                        "gam": ma.tile([P, NCH], F32, tag="gam", name="gam"),
                        "acc": ma.tile([P, dvp], F32, tag="acc", name="acc"),
                    }
                    stats.append(st)

                import collections
                pend = collections.deque()  # (k, pt_c) with deep PV lag
                tri = 0  # xbar transpose engine alternator

                def do_scores(k):
                    nonlocal tri
                    ch, it = divmod(k, IT)
                    st = stats[it]
                    ps = sps.tile([P, chunk], F32, tag="s", name="ps")
                    for ct in range(CT):
                        for nn in range(NN):
                            nc.tensor.matmul(
                                ps[:, nn * W512:(nn + 1) * W512],
                                qt_s[:, ct, it * P:(it + 1) * P],
                                (kt_loc[:, ct, nn * W512:(nn + 1) * W512]
                                 if ch == 0 else
                                 kt_full[
                                     :, ch - 1, ct,
                                     nn * W512:(nn + 1) * W512,
                                 ]),
                                start=(ct == 0),
                                stop=(ct == CT - 1),
                                skip_group_check=True,
                            )
                    nc.vector.reduce_max(
                        st["mneg"][:, ch:ch + 1], ps[:],
                        axis=mybir.AxisListType.X, negate=True,
                    )
                    if ch > 0:
                        nc.vector.tensor_tensor(
                            st["nmr"][:, ch:ch + 1], st["nmr"][:, ch - 1:ch],
                            st["mneg"][:, ch:ch + 1], op=mybir.AluOpType.min,
                        )
                    else:
                        nc.vector.tensor_copy(st["nmr"][:, :1], st["mneg"][:, :1])
                    p_c = cp.tile([P, chunk], BF16, tag="p", name="p_c", bufs=16)
                    # bias = running max -> P is exp(s - m_run(ch)); no
                    # beta, and no accum_out: the denominator comes from
                    # the PV ones-column
                    nc.scalar.activation(
                        p_c[:], ps[:],
                        mybir.ActivationFunctionType.Exp,
                        bias=st["nmr"][:, ch:ch + 1],
                        scale=1.0,
                    )
                    if ch < 2:
                        # xbar transposes are serialized against in-flight
                        # collectives; ch0/ch1 (which overlap the K and V
                        # AllGathers) must transpose on the PE instead
                        return p_c
                    pt_c = cp.tile([P, JPC, P], BF16, tag="pt", name="pt_c", bufs=30)
                    # DMA_TRANSPOSE burns ~1.3us of ISSUING-ENGINE time;
                    # keep them all on sync (only SP/Activation are
                    # HWDGE-capable, and scalar is busy with exps).
                    # ch2/ch3's transposes queue behind the v-anchor in
                    # sync's stream, which keeps the xbar (hardware-
                    # serialized against in-flight collectives) away
                    # from the V AllGather.
                    nc.sync.dma_start_transpose(pt_c[:], p_c[:])
                    return pt_c

                def do_tr_pe(p_c_t):
                    # P^T via PE for ch0/ch1 (PSUM is full: borrow a
                    # scores-ring tile, pack the 8 bf16 transpose outputs
                    # into its first half via bitcast slices)
                    pt_c = cp.tile([P, JPC, P], BF16, tag="pt", name="pt_c", bufs=30)
                    ps_tr = sps.tile([P, chunk], F32, tag="s", name="ps_tr")
                    for j2 in range(JPC):
                        tpp = ps_tr[:, j2 * 64:(j2 + 1) * 64].bitcast(BF16)
                        nc.tensor.transpose(
                            tpp, p_c_t[:, j2 * P:(j2 + 1) * P], ident_b[:]
                        )
                        if j2 % 2:
                            nc.vector.tensor_copy(pt_c[:, j2, :], tpp)
                        else:
                            nc.scalar.copy(pt_c[:, j2, :], tpp)
                    return pt_c

                def do_pv(k, pt_c):
                    ch, it = divmod(k, IT)
                    st = stats[it]
                    po = ops.tile([P, dvp], F32, tag="po", name="po")
                    for j2 in range(JPC):
                        nc.tensor.matmul(
                            po[:], pt_c[:, j2, :],
                            (v_loc[:, j2, :] if ch == 0 else
                             v_s[:, (ch - 1) * JPC + j2, :]),
                            start=(j2 == 0), stop=(j2 == JPC - 1),
                        )
                    if ch == 0:
                        nc.vector.tensor_copy(st["acc"][:], po[:])
                    else:
                        # gamma = exp(m_run(ch-1) - m_run(ch))
                        nc.scalar.activation(
                            st["gam"][:, ch:ch + 1], st["nmr"][:, ch - 1:ch],
                            mybir.ActivationFunctionType.Exp,
                            bias=st["nmr"][:, ch:ch + 1], scale=-1.0,
                        )
                        # acc = acc*gamma + po (P already carries beta via
                        # the running-max exp bias)
                        nc.vector.scalar_tensor_tensor(
                            st["acc"][:], st["acc"][:], st["gam"][:, ch:ch + 1],
                            po[:],
                            op0=mybir.AluOpType.mult,
                            op1=mybir.AluOpType.add,
                        )
                    if ch == NCH - 1:
                        # denominator rode along in the ones-column
                        nc.vector.reciprocal(
                            st["rinv"][:], st["acc"][:, dv:dvp]
                        )
                        nc.vector.tensor_scalar_mul(
                            st["acc"][:, :dv], st["acc"][:, :dv], st["rinv"][:]
                        )
                        nc.sync.dma_start(
                            out_ext.ap().rearrange("(it p) c -> p it c", p=P)[
                                :, it, :
                            ],
                            st["acc"][:, :dv],
                        )

                order = [g * IT + i2 for g in range(NCH) for i2 in range(IT)]
                LAG = min(28, max(1, len(order) - 1))
                LAG0 = 3  # shallow lag inside ch0 so it completes in-AG
                anchor = ma.tile([2, 64], U16, tag="anchor", name="anchor",
                                 bufs=2)

                # NOTE: Tile rotates hardware DMAs over 8 completion
                # semaphores in EMISSION order; a DMA must wait for its
                # lane's previous occupant. AG-gated assembly DMAs must
                # therefore be emitted AFTER all of ch0's transposes, or
                # ch0 (which is AG-independent) transitively waits on the
                # collective.
                TRPE = 2 * IT  # chunks with PE transposes (AG overlap)
                sc_pend = collections.deque()  # (k, p_c) awaiting PE tr
                for k in order:
                    if k == IT:
                        # ch0 fully emitted; flush its transposes + PVs
                        # ahead of the first AG1-dependent scores (ch1
                        # PVs stay queued: they need the V AllGather)
                        while sc_pend:
                            kk, pc_t = sc_pend.popleft()
                            pend.append((kk, do_tr_pe(pc_t)))
                        while pend:
                            kk, pt = pend.popleft()
                            do_pv(kk, pt)
                        # kt assembly first (scores ch1 needs slot 1
                        # before PV needs V). anchor = static-offset
                        # pack_ag read carrying the collective wait for
                        # the dynamic-offset DMAs, which Tile can't
                        # track. tile_wait_until keeps the scheduler from
                        # hoisting these AG-gated DMAs ahead of ch0's
                        # transposes/exps in the engine streams (which
                        # parks those engines on the collective).
                        # everything on the SYNC engine: it is idle at
                        # AG1-completion, so assembly starts immediately
                        # slot 1 gets sync's DMA ring to itself (its
                        # packets otherwise interleave behind all six
                        # other slots + the in-flight AG2 traffic and
                        # land ~9us late, stalling ch1 scores); slots
                        # 2-7 go via scalar's ring (scalar is idle at
                        # AG1-completion, with its own anchor + rank
                        # register).
                        with tc.tile_wait_until(1.0):
                            nc.sync.dma_start(
                                anchor[:1, :], pack_k_ag[:][1:2, 0:64]
                            )
                            rk_sync = nc.sync.cc_rank(groups)
                            assemble_kt(1, rk_sync, nc.sync)
                            nc.scalar.dma_start(
                                anchor[1:2, :], pack_k_ag[:][2:3, 0:64]
                            )
                            rk_act = nc.scalar.cc_rank(groups)
                            for s in range(2, NCH):
                                assemble_kt(s, rk_act, nc.scalar)
                            nc.sync.dma_start(
                                anchor[:1, :], pack_v_ag[:][0:1, 0:64]
                            )
                            for s in range(1, NCH):
                                assemble_v(s, rk_sync)
                    if k == TRPE:
                        # last PE-transposed chunk flushes before the
                        # first xbar-transposed one
                        while sc_pend:
                            kk, pc_t = sc_pend.popleft()
                            pend.append((kk, do_tr_pe(pc_t)))
                    res = do_scores(k)
                    if k < TRPE:
                        # PE-transpose lags scores by one chunk so the
                        # in-order PE never stalls on the exp chain
                        sc_pend.append((k, res))
                        if len(sc_pend) > 1:
                            kk, pc_t = sc_pend.popleft()
                            pend.append((kk, do_tr_pe(pc_t)))
                    else:
                        pend.append((k, res))
                    lag = LAG0 if k < IT else LAG
                    if len(pend) > lag:
                        kk, pt = pend.popleft()
                        do_pv(kk, pt)
                    # once every PV's V-slot is safely assembled (AG2 done
                    # well before ch4), amortize the deep lag down so the
                    # final drain isn't ~LAG serial PVs on the PE
                    if k >= 4 * IT and len(pend) > 8:
                        kk, pt = pend.popleft()
                        do_pv(kk, pt)
                while pend:
                    kk, pt = pend.popleft()
                    do_pv(kk, pt)

            phb_cm.__exit__(None, None, None)

    nc.finalize()
    return nc


_NC_CACHE = {}


def _get_nc(key):
    if key not in _NC_CACHE:
        n, d, dqk, dv, ncores = key
        _NC_CACHE[key] = build(n=n, d=d, dqk=dqk, dv=dv, ncores=ncores)
    return _NC_CACHE[key]


def run(x, Wq, Wk, Wv, trace=False):
    n, d = x.shape
    dqk = Wq.shape[0]
    dv = Wv.shape[0]
    ncores = N_CORES
    nsh = n // ncores
    nc = _get_nc((n, d, dqk, dv, ncores))

    x = np.ascontiguousarray(x, dtype=np.float32)
    Wq = np.ascontiguousarray(Wq, dtype=np.float32)
    Wk = np.ascontiguousarray(Wk, dtype=np.float32)
    Wv = np.ascontiguousarray(Wv, dtype=np.float32)

    in_maps = [
        {"x": x[r * nsh:(r + 1) * nsh], "Wq": Wq, "Wk": Wk, "Wv": Wv}
        for r in range(ncores)
    ]
    res = run_bass_kernel_spmd(
        nc, in_maps, core_ids=list(range(ncores)), trace=trace
    )
    out = np.concatenate([res.results[r]["out"] for r in range(ncores)], axis=0)
    return out, res


def kernel(x, Wq, Wk, Wv):
    out, _ = run(x, Wq, Wk, Wv)
    return out


# revision 15
# speedup vs baseline: 1.0692x; 1.0364x over previous
"""Distributed attention kernel for TRN2 (8 NeuronCores), v2.

Computes: softmax(sqrt(Dqk) * (x@Wq.T) @ (x@Wk.T).T) @ (x@Wv.T)
for x [8192, 1024], Wq/Wk/Wv [256, 1024], out [8192, 256].

Sharding: rows of x across 8 cores (sequence parallel), weights
replicated; K^T/V AllGathered; flash-style attention over each core's
1024 Q rows.

v2 changes over the 286-292us v1 baseline (see kernel_v1_baseline.py
for the v1 rationale docstring; all of it still applies):
  - warmup: a tiny dummy AllGather fires at kernel start. The first
    real collective in v1 paid a ~58us BARRIER (launch skew across the
    8 PJRT dispatches + CC-stream bootstrap); the dummy absorbs it
    concurrently with Phase A compute.
  - Phase A in f16: x is cast f32->f16 once per i-tile, all x^T/W^T
    PE transposes run in f16 (packed 4-to-a-PSUM-tile, single eviction
    per group), and the q/k/v projections are f16 (full PE rate vs
    2-pass f32r). CPU sim: adds ~1.5e-3 rel err (4.6e-3 -> ~7e-3 on
    HW, gate 2e-2). Cuts Phase A PE time roughly in half, so pack_k
    staging (which gates AG1) lands earlier.
  - q-projection moved BEFORE v-projection: ch0 scores only need q,
    and AG2 serializes behind AG1 on the CC stream anyway, so v can
    stage late without delaying AG2.
  - 4KB-packet assembly: pack_k is [P, CT*nsh] (per-partition 4KB
    contiguous rows), kt_full is [P, NCH-1, CT, nsh]; pack_v carries
    the ones-column in the payload (dvp=257 rows) so v_s slot writes
    are 4112B-contiguous. v1's 2KB/512B packets ran the slot-1
    assembly at ~50GB/s (10.3us!) which stalled ch1 scores.
  - xbar P^T transposes alternate sync/gpsimd engines (each costs
    ~1.3us of issuing-engine time; sync also carries assembly + out).
"""

import numpy as np

import concourse.bacc as bacc
import concourse.bass as bass
import concourse.mybir as mybir
import concourse.tile as tile
from concourse.bass_utils import run_bass_kernel_spmd
from concourse.masks import make_identity

F32 = mybir.dt.float32
BF16 = mybir.dt.bfloat16
F16 = mybir.dt.float16
U16 = mybir.dt.uint16

N_CORES = 8
N, D, DQK, DV = 8192, 1024, 256, 256
P = 128
CHUNK = 1024  # scores chunk width (2 PSUM banks)


def build(n=N, d=D, dqk=DQK, dv=DV, ncores=N_CORES):
    nsh = n // ncores
    IT = nsh // P       # i-tiles per core
    KT = d // P         # contract tiles for projections
    CT = dqk // P       # dqk tiles
    chunk = min(CHUNK, nsh)
    assert chunk == nsh, "rotation requires one chunk per rank"
    NCH = n // chunk    # score chunks per row == ncores
    W512 = min(512, chunk)
    NN = chunk // W512  # W512-wide matmuls per chunk
    scale = float(np.sqrt(dqk))
    dvp = dv + 1  # ones-column rides along: PV accumulates the softmax
    # denominator with all gamma corrections for free

    nc = bacc.Bacc(None, target_bir_lowering=False, num_devices=ncores)

    x_ext = nc.declare_dram_parameter("x", [nsh, d], F32, isOutput=False)
    wq_ext = nc.declare_dram_parameter("Wq", [dqk, d], F32, isOutput=False)
    wk_ext = nc.declare_dram_parameter("Wk", [dqk, d], F32, isOutput=False)
    wv_ext = nc.declare_dram_parameter("Wv", [dv, d], F32, isOutput=False)
    out_ext = nc.declare_dram_parameter("out", [nsh, dv], F32, isOutput=True)

    groups = [list(range(ncores))]

    with tile.TileContext(nc) as tc:
        with (
            tc.tile_pool(name="persist", bufs=1) as pp,
            tc.tile_pool(name="dramp", bufs=1, space="DRAM") as dp,
        ):
            # warmup collective: absorbs launch-skew / CC bootstrap so
            # the real AG1 doesn't pay a long barrier
            warm = dp.tile([1, 64], U16, name="warm")
            warm_ag = dp.tile([ncores, 64], U16, addr_space="Shared",
                              name="warm_ag")
            nc.gpsimd.collective_compute(
                "AllGather",
                mybir.AluOpType.bypass,
                replica_groups=groups,
                ins=[warm[:].opt()],
                outs=[warm_ag[:].opt()],
            )

            # Packed AG buffers. Per-partition rows are 4KB-contiguous
            # so the assembly DMAs move 4KB packets:
            #   pack_k: partition p holds k^T rows (p, 128+p) = CT*nsh
            #           f16 = 4KB — AllGathered FIRST (gates scores)
            #   pack_v: v as [IT, dvp] bf16 slabs (ones-column packed
            #           in!) split over rows 2p, 2p+1 — gathered second
            pack_k = dp.tile([P, CT * nsh], U16, name="pack_k")
            pack_k_ag = dp.tile(
                [ncores * P, CT * nsh], U16, addr_space="Shared",
                name="pack_k_ag",
            )
            VROW = IT * dvp // 2  # 1028 u16 per packed v row
            pack_v = dp.tile([2 * P, VROW], U16, name="pack_v")
            pack_v_ag = dp.tile(
                [ncores * 2 * P, VROW], U16, addr_space="Shared",
                name="pack_v_ag",
            )

            qt_s = pp.tile([P, CT, nsh], F16, tag="qt")
            kt_loc = pp.tile([P, CT, nsh], F16, tag="ktloc")
            v_loc = pp.tile([P, IT, dvp], BF16, tag="vloc")
            nc.vector.memset(v_loc[:, :, dv:dvp], 1.0)
            ident_h = pp.tile([P, P], F16, tag="identh")
            make_identity(nc, ident_h[:])
            ident_b = pp.tile([P, P], BF16, tag="identb")
            make_identity(nc, ident_b[:])

            # ================= Phase A =================
            with (
                tc.tile_pool(name="phA", bufs=1) as pa,
                tc.tile_pool(name="phA_psum", bufs=1, space="PSUM") as paps,
            ):
                x_nat = pa.tile([P, IT, d], F32, tag="xnat")
                xh = pa.tile([P, IT, d], F16, tag="xh")
                w_nat = pa.tile([P, 3 * CT, d], F32, tag="wnat")
                w_exts = {0: wq_ext, 1: wk_ext, 2: wv_ext}

                def w_dma(wi):
                    nc.scalar.dma_start(
                        w_nat[:, wi * CT:(wi + 1) * CT, :],
                        w_exts[wi].ap().rearrange("(ct p) d -> p ct d", p=P),
                    )

                def x_dma(it, eng):
                    eng.dma_start(
                        x_nat[:, it, :],
                        x_ext.ap().rearrange("(it p) d -> p it d", p=P)[:, it, :],
                    )

                # scalar queue: Wk FIRST (it gates the Wk transposes ->
                # k-proj -> AG1 path); x evens on sync in parallel
                w_dma(1)
                for it in range(0, IT, 2):
                    x_dma(it, nc.sync)
                for it in range(1, IT, 2):
                    x_dma(it, nc.scalar)
                w_dma(0)
                w_dma(2)

                # cast x to f16 as tiles land (transposes + projections
                # run in f16 — halves PE cycles vs f32)
                for it in range(IT):
                    if it % 2:
                        nc.scalar.copy(xh[:, it, :], x_nat[:, it, :])
                    else:
                        nc.vector.tensor_copy(xh[:, it, :], x_nat[:, it, :])

                xt_s = pa.tile([P, KT, nsh], F16, tag="xt")
                wt_s = pa.tile([P, 3 * KT, dqk], F16, tag="wt")

                ei = 0  # eviction engine alternator

                def evict(dst, src):
                    nonlocal ei
                    ei += 1
                    if ei % 2:
                        nc.vector.tensor_copy(dst, src)
                    else:
                        nc.scalar.copy(dst, src)

                ich_w = min(512, nsh)

                # f16 transposes packed 4-per-PSUM-tile with one
                # strided eviction per group (vs v1's per-tile evicts)
                def x_transposes(it_range):
                    for it in it_range:
                        for g in range(KT // 4):
                            tp = paps.tile([P, 256], F32, tag="tp", bufs=4)
                            tph = tp[:].bitcast(F16)  # [P, 512]
                            for j in range(4):
                                kt = 4 * g + j
                                nc.tensor.transpose(
                                    tph[:, j * P:(j + 1) * P],
                                    xh[:, it, kt * P:(kt + 1) * P],
                                    ident_h[:],
                                )
                            evict(
                                xt_s[:, 4 * g:4 * g + 4, it * P:(it + 1) * P],
                                tph.rearrange("p (f i) -> p f i", f=4),
                            )

                def w_transposes(wi):
                    # cast this W to f16 in-place groups then transpose:
                    # pack (kt pair x ct pair) -> contiguous 512 dst
                    wh = pa.tile([P, CT, d], F16, tag=f"wh{wi}")
                    nc.vector.tensor_copy(wh[:, :, :], w_nat[:, wi * CT:(wi + 1) * CT, :])
                    for g in range(KT // 2):
                        tp = paps.tile([P, 256], F32, tag="tp", bufs=4)
                        tph = tp[:].bitcast(F16)  # [P, 512]
                        for j in range(2):
                            kt = 2 * g + j
                            for ct in range(CT):
                                nc.tensor.transpose(
                                    tph[:, (j * CT + ct) * P:(j * CT + ct + 1) * P],
                                    wh[:, ct, kt * P:(kt + 1) * P],
                                    ident_h[:],
                                )
                        evict(
                            wt_s[:, wi * KT + 2 * g:wi * KT + 2 * g + 2, :],
                            tph.rearrange("p (f i) -> p f i", f=2),
                        )

                def k_proj(ich):
                    for ct in range(CT):
                        psk = paps.tile([P, ich_w], F32, tag="pqk", bufs=2)
                        for kt in range(KT):
                            nc.tensor.matmul(
                                psk[:],
                                wt_s[:, 1 * KT + kt, ct * P:(ct + 1) * P],
                                xt_s[:, kt, ich * ich_w:(ich + 1) * ich_w],
                                start=(kt == 0),
                                stop=(kt == KT - 1),
                            )
                        nc.vector.tensor_copy(
                            kt_loc[:, ct, ich * ich_w:(ich + 1) * ich_w],
                            psk[:],
                        )
                    # stage this ich half immediately (AG waits on staging)
                    nc.sync.dma_start(
                        pack_k[:].bitcast(F16).rearrange(
                            "p (ct i) -> p ct i", ct=CT
                        )[:, :, ich * ich_w:(ich + 1) * ich_w],
                        kt_loc[:, :, ich * ich_w:(ich + 1) * ich_w],
                    )

                # k_proj(ich) only needs x^T of i-rows in that half:
                # interleave so AG1 triggers as early as possible
                ipw = ich_w // P
                x_transposes(range(ipw))
                w_transposes(1)           # Wk
                k_proj(0)
                x_transposes(range(ipw, IT))
                k_proj(1)
                # K AllGather fires as soon as k^T is staged; scores for
                # ch>=1 depend only on this one
                nc.gpsimd.collective_compute(
                    "AllGather",
                    mybir.AluOpType.bypass,
                    replica_groups=groups,
                    ins=[pack_k[:].opt()],
                    outs=[pack_k_ag[:].opt()],
                )

                # Wq transposes + q^T projection next: ch0 scores only
                # need q, and AG2 serializes behind AG1 on the CC
                # stream anyway so v can stage later without cost.
                # ich outer so q rows 0-511 finish first.
                w_transposes(0)
                for ich in range(nsh // ich_w):
                    for ct in range(CT):
                        psq = paps.tile([P, ich_w], F32, tag="pqk", bufs=2)
                        for kt in range(KT):
                            nc.tensor.matmul(
                                psq[:],
                                wt_s[:, 0 * KT + kt, ct * P:(ct + 1) * P],
                                xt_s[:, kt, ich * ich_w:(ich + 1) * ich_w],
                                start=(kt == 0),
                                stop=(kt == KT - 1),
                            )
                        nc.vector.tensor_scalar_mul(
                            qt_s[:, ct, ich * ich_w:(ich + 1) * ich_w],
                            psq[:],
                            scale,
                        )

                w_transposes(2)           # Wv
                # v projections
                for it in range(IT):
                    psv = paps.tile([P, dv], F32, tag="psv", bufs=2)
                    for kt in range(KT):
                        nc.tensor.matmul(
                            psv[:],
                            xt_s[:, kt, it * P:(it + 1) * P],
                            wt_s[:, 2 * KT + kt, :dqk],
                            start=(kt == 0),
                            stop=(kt == KT - 1),
                        )
                    nc.vector.tensor_copy(v_loc[:, it, :dv], psv[:])
                # v packed WITH the ones column: partition p's [IT, dvp]
                # slab -> rows 2p, 2p+1 (so receiver slot writes are one
                # contiguous 4112B run per partition)
                nc.sync.dma_start(
                    pack_v[:].bitcast(BF16).rearrange(
                        "(p r) i -> p (r i)", p=P
                    ),
                    v_loc[:, :, :],
                )
                # V AllGather (runs after the K one on the CC stream)
                nc.gpsimd.collective_compute(
                    "AllGather",
                    mybir.AluOpType.bypass,
                    replica_groups=groups,
                    ins=[pack_v[:].opt()],
                    outs=[pack_v_ag[:].opt()],
                )

            phb_cm = tc.tile_pool(name="phB", bufs=1)
            phb = phb_cm.__enter__()
            # slots 1..NCH-1 only (slot 0 is local kt_loc / v_loc)
            kt_full = phb.tile([P, NCH - 1, CT, nsh], F16, tag="ktf",
                               name="kt_full")
            v_s = phb.tile([P, (NCH - 1) * IT, dvp], BF16, tag="vs",
                           name="v_s")

            pag_h = pack_k_ag[:].bitcast(F16)
            pag_b = pack_v_ag[:].bitcast(BF16)

            def assemble_v(s, rk):
                # slot s holds rank (my_rank + s) % ncores
                vrow = ((rk + s) % ncores) * 2 * P
                nc.sync.dma_start(
                    v_s[:, (s - 1) * IT:s * IT, :],
                    pag_b[bass.ds(vrow, 2 * P), :].rearrange(
                        "(p r) i -> p (r i)", p=P
                    ),
                )

            def assemble_kt(s, rk, eng):
                row = ((rk + s) % ncores) * P
                eng.dma_start(
                    kt_full[:, s - 1, :, :],
                    pag_h[bass.ds(row, P), :].rearrange(
                        "p (ct i) -> p ct i", ct=CT
                    ),
                )

            # ================= Main attention loop =================
            # Streaming flash-attention: each 1024-wide chunk flows
            # MM -> row-max(DVE) -> exp(ACT, running-max bias) -> P^T
            # transpose -> PV -> one-op accumulator merge.
            with (
                tc.tile_pool(name="mainA", bufs=8) as ma,
                tc.tile_pool(name="chunkp", bufs=2) as cp,
                tc.tile_pool(name="scores_psum", bufs=3, space="PSUM") as sps,
                tc.tile_pool(name="out_psum", bufs=2, space="PSUM") as ops,
            ):
                JPC = chunk // P  # j-tiles per chunk (== IT)

                NTOT = IT * NCH
                stats = []
                for it in range(IT):
                    st = {
                        "mneg": ma.tile([P, NCH], F32, tag="mneg", name="mneg"),
                        "nmr": ma.tile([P, NCH], F32, tag="nmr", name="nmr"),
                        "rinv": ma.tile([P, 1], F32, tag="rinv", name="rinv"),
                        "gam": ma.tile([P, NCH], F32, tag="gam", name="gam"),
                        "acc": ma.tile([P, dvp], F32, tag="acc", name="acc"),
                    }
                    stats.append(st)

                import collections
                pend = collections.deque()  # (k, pt_c) with deep PV lag
                tri = 0  # xbar transpose engine alternator

                def do_scores(k):
                    nonlocal tri
                    ch, it = divmod(k, IT)
                    st = stats[it]
                    ps = sps.tile([P, chunk], F32, tag="s", name="ps")
                    for ct in range(CT):
                        for nn in range(NN):
                            nc.tensor.matmul(
                                ps[:, nn * W512:(nn + 1) * W512],
                                qt_s[:, ct, it * P:(it + 1) * P],
                                (kt_loc[:, ct, nn * W512:(nn + 1) * W512]
                                 if ch == 0 else
                                 kt_full[
                                     :, ch - 1, ct,
                                     nn * W512:(nn + 1) * W512,
                                 ]),
                                start=(ct == 0),
                                stop=(ct == CT - 1),
                                skip_group_check=True,
                            )
                    nc.vector.reduce_max(
                        st["mneg"][:, ch:ch + 1], ps[:],
                        axis=mybir.AxisListType.X, negate=True,
                    )
                    if ch > 0:
                        nc.vector.tensor_tensor(
                            st["nmr"][:, ch:ch + 1], st["nmr"][:, ch - 1:ch],
                            st["mneg"][:, ch:ch + 1], op=mybir.AluOpType.min,
                        )
                    else:
                        nc.vector.tensor_copy(st["nmr"][:, :1], st["mneg"][:, :1])
                    p_c = cp.tile([P, chunk], BF16, tag="p", name="p_c", bufs=24)
                    # bias = running max -> P is exp(s - m_run(ch)); no
                    # beta, and no accum_out: the denominator comes from
                    # the PV ones-column
                    nc.scalar.activation(
                        p_c[:], ps[:],
                        mybir.ActivationFunctionType.Exp,
                        bias=st["nmr"][:, ch:ch + 1],
                        scale=1.0,
                    )
                    if k < TRPE:
                        # xbar transposes are serialized against in-flight
                        # collectives; ch0 (which overlaps the K AllGather)
                        # must transpose on the PE instead. ch1+ xbar
                        # transposes queue behind the v-anchor on sync, so
                        # they only run once AG2 has completed.
                        return p_c
                    pt_c = cp.tile([P, JPC, P], BF16, tag="pt", name="pt_c", bufs=30)
                    # DMA_TRANSPOSE burns ~1.3us of ISSUING-ENGINE time;
                    # keep them all on sync (only SP/Activation are
                    # HWDGE-capable, and scalar is busy with exps).
                    # ch2/ch3's transposes queue behind the v-anchor in
                    # sync's stream, which keeps the xbar (hardware-
                    # serialized against in-flight collectives) away
                    # from the V AllGather.
                    nc.sync.dma_start_transpose(pt_c[:], p_c[:])
                    return pt_c

                def do_tr_pe(p_c_t):
                    # P^T via PE for ch0/ch1 (PSUM is full: borrow a
                    # scores-ring tile, pack the 8 bf16 transpose outputs
                    # into its first half via bitcast slices)
                    pt_c = cp.tile([P, JPC, P], BF16, tag="pt", name="pt_c", bufs=30)
                    ps_tr = sps.tile([P, chunk], F32, tag="s", name="ps_tr")
                    for j2 in range(JPC):
                        tpp = ps_tr[:, j2 * 64:(j2 + 1) * 64].bitcast(BF16)
                        nc.tensor.transpose(
                            tpp, p_c_t[:, j2 * P:(j2 + 1) * P], ident_b[:]
                        )
                        if j2 % 2:
                            nc.vector.tensor_copy(pt_c[:, j2, :], tpp)
                        else:
                            nc.scalar.copy(pt_c[:, j2, :], tpp)
                    return pt_c

                def do_pv(k, pt_c):
                    ch, it = divmod(k, IT)
                    st = stats[it]
                    po = ops.tile([P, dvp], F32, tag="po", name="po")
                    for j2 in range(JPC):
                        nc.tensor.matmul(
                            po[:], pt_c[:, j2, :],
                            (v_loc[:, j2, :] if ch == 0 else
                             v_s[:, (ch - 1) * JPC + j2, :]),
                            start=(j2 == 0), stop=(j2 == JPC - 1),
                        )
                    if ch == 0:
                        nc.vector.tensor_copy(st["acc"][:], po[:])
                    else:
                        # gamma = exp(m_run(ch-1) - m_run(ch))
                        nc.scalar.activation(
                            st["gam"][:, ch:ch + 1], st["nmr"][:, ch - 1:ch],
                            mybir.ActivationFunctionType.Exp,
                            bias=st["nmr"][:, ch:ch + 1], scale=-1.0,
                        )
                        # acc = acc*gamma + po (P already carries beta via
                        # the running-max exp bias)
                        nc.vector.scalar_tensor_tensor(
                            st["acc"][:], st["acc"][:], st["gam"][:, ch:ch + 1],
                            po[:],
                            op0=mybir.AluOpType.mult,
                            op1=mybir.AluOpType.add,
                        )
                    if ch == NCH - 1:
                        # denominator rode along in the ones-column
                        nc.vector.reciprocal(
                            st["rinv"][:], st["acc"][:, dv:dvp]
                        )
                        nc.vector.tensor_scalar_mul(
                            st["acc"][:, :dv], st["acc"][:, :dv], st["rinv"][:]
                        )
                        nc.sync.dma_start(
                            out_ext.ap().rearrange("(it p) c -> p it c", p=P)[
                                :, it, :
                            ],
                            st["acc"][:, :dv],
                        )

                order = [g * IT + i2 for g in range(NCH) for i2 in range(IT)]
                LAG = min(28, max(1, len(order) - 1))
                LAG0 = 3  # shallow lag inside ch0 so it completes in-AG
                anchor = ma.tile([2, 64], U16, tag="anchor", name="anchor",
                                 bufs=2)

                # NOTE: Tile rotates hardware DMAs over 8 completion
                # semaphores in EMISSION order; a DMA must wait for its
                # lane's previous occupant. AG-gated assembly DMAs must
                # therefore be emitted AFTER all of ch0's transposes, or
                # ch0 (which is AG-independent) transitively waits on the
                # collective.
                # Only ch0 transposes on the PE. ch1's xbar transposes
                # queue behind the v-anchor on sync, so they run right
                # after AG2 completes — and ch1's PVs pop from the deep
                # LAG queue even later, so nothing stalls. This removes
                # ~22us of PE work from the heavily-throttled post-AG1
                # window (v1/v2 ran ch1 on the PE too).
                TRPE = IT
                sc_pend = collections.deque()  # (k, p_c) awaiting PE tr
                for k in order:
                    if k == IT:
                        # ch0 fully emitted; flush its transposes + PVs
                        # ahead of the first AG1-dependent scores (ch1
                        # PVs stay queued: they need the V AllGather)
                        while sc_pend:
                            kk, pc_t = sc_pend.popleft()
                            pend.append((kk, do_tr_pe(pc_t)))
                        while pend:
                            kk, pt = pend.popleft()
                            do_pv(kk, pt)
                        # kt assembly first (scores ch1 needs slot 1
                        # before PV needs V). anchor = static-offset
                        # pack_ag read carrying the collective wait for
                        # the dynamic-offset DMAs, which Tile can't
                        # track. tile_wait_until keeps the scheduler from
                        # hoisting these AG-gated DMAs ahead of ch0's
                        # transposes/exps in the engine streams (which
                        # parks those engines on the collective).
                        # everything on the SYNC engine: it is idle at
                        # AG1-completion, so assembly starts immediately
                        # ALL assembly on the sync engine: the 8 DMA
                        # completion-semaphore lanes are GLOBAL across
                        # engines and rotate in emission order, so
                        # splitting slots onto scalar chains scalar's
                        # kt DMAs behind sync's AG2-gated v DMAs (lane
                        # collision) and parks every ch1+ exp on AG2
                        # (measured: exps at 142us instead of 126us).
                        with tc.tile_wait_until(1.0):
                            nc.sync.dma_start(
                                anchor[:1, :], pack_k_ag[:][1:2, 0:64]
                            )
                            rk_sync = nc.sync.cc_rank(groups)
                            for s in range(1, NCH):
                                assemble_kt(s, rk_sync, nc.sync)
                            nc.sync.dma_start(
                                anchor[:1, :], pack_v_ag[:][0:1, 0:64]
                            )
                            for s in range(1, NCH):
                                assemble_v(s, rk_sync)
                    if k == TRPE:
                        # last PE-transposed chunk flushes before the
                        # first xbar-transposed one
                        while sc_pend:
                            kk, pc_t = sc_pend.popleft()
                            pend.append((kk, do_tr_pe(pc_t)))
                    res = do_scores(k)
                    if k < TRPE:
                        # PE-transpose lags scores by one chunk so the
                        # in-order PE never stalls on the exp chain
                        sc_pend.append((k, res))
                        if len(sc_pend) > 1:
                            kk, pc_t = sc_pend.popleft()
                            pend.append((kk, do_tr_pe(pc_t)))
                    else:
                        pend.append((k, res))
                    lag = LAG0 if k < IT else LAG
                    if len(pend) > lag:
                        kk, pt = pend.popleft()
                        do_pv(kk, pt)
                    # once every PV's V-slot is safely assembled (AG2 done
                    # well before ch4), amortize the deep lag down so the
                    # final drain isn't ~LAG serial PVs on the PE
                    if k >= 4 * IT and len(pend) > 8:
                        kk, pt = pend.popleft()
                        do_pv(kk, pt)
                    # late-stage: drain harder so the post-loop tail
                    # isn't a long serial PV chain on the PE
                    if k >= 6 * IT and len(pend) > 4:
                        kk, pt = pend.popleft()
                        do_pv(kk, pt)
                while pend:
                    kk, pt = pend.popleft()
                    do_pv(kk, pt)

            phb_cm.__exit__(None, None, None)

    nc.finalize()
    return nc


_NC_CACHE = {}


def _get_nc(key):
    if key not in _NC_CACHE:
        n, d, dqk, dv, ncores = key
        _NC_CACHE[key] = build(n=n, d=d, dqk=dqk, dv=dv, ncores=ncores)
    return _NC_CACHE[key]


def run(x, Wq, Wk, Wv, trace=False):
    n, d = x.shape
    dqk = Wq.shape[0]
    dv = Wv.shape[0]
    ncores = N_CORES
    nsh = n // ncores
    nc = _get_nc((n, d, dqk, dv, ncores))

    x = np.ascontiguousarray(x, dtype=np.float32)
    Wq = np.ascontiguousarray(Wq, dtype=np.float32)
    Wk = np.ascontiguousarray(Wk, dtype=np.float32)
    Wv = np.ascontiguousarray(Wv, dtype=np.float32)

    in_maps = [
        {"x": x[r * nsh:(r + 1) * nsh], "Wq": Wq, "Wk": Wk, "Wv": Wv}
        for r in range(ncores)
    ]
    res = run_bass_kernel_spmd(
        nc, in_maps, core_ids=list(range(ncores)), trace=trace
    )
    out = np.concatenate([res.results[r]["out"] for r in range(ncores)], axis=0)
    return out, res


def kernel(x, Wq, Wk, Wv):
    out, _ = run(x, Wq, Wk, Wv)
    return out
